# revision 1
# baseline (speedup 1.0000x reference)
"""Trainium2 Bass kernel for nn_CustomMLPLayer_74526272520565 (topk_masking).

Reference semantics:
  core_idx = top-n_core neurons by how often they appear in each token's
             top-k_tok activations (count ties broken toward lower index)
  out = x[..., core_idx] @ W[:, core_idx].T

Distribution (8 NeuronCores): tensor-parallel on W rows (output dim),
x replicated; the core-neuron counts are token-sharded and AllReduced.

Per-core device algorithm:
  A. For its 256-token slice: exact k_tok-th largest activation per token via
     dyadic bisection on count(x > t) (fused compare+accumulate probes split
     across VectorE and ScalarE), finished by a top-8 + rank-select step.
     sel = (x >= t*); counts[j] = sum_s sel[s, j] via PE matmuls.
  B. AllReduce counts; exact core-set threshold: integer bisection for the
     count threshold tau, then index bisection among count==tau ties.
  C. Compact the 4403 core indices (gpsimd sparse_gather) + 77 zero-row pads.
  D. dma_gather the core rows of host-pre-transposed f16 x^T [H, S] and
     W^T shard [H, 512]; reduced GEMM (K=4480) accumulated in PSUM f32.
"""
import numpy as np

import concourse.bass as bass
import concourse.mybir as mybir
from concourse.tile import TileContext
from concourse.tile_rust import add_dep_helper
from concourse import library_config
from concourse.bass_utils import run_bass_kernel_spmd

AF = mybir.ActivationFunctionType
OP = mybir.AluOpType
F32 = mybir.dt.float32
F16 = mybir.dt.float16
U8 = mybir.dt.uint8
I16 = mybir.dt.int16
U32 = mybir.dt.uint32

N_CORES = 8

REAL = dict(S=2048, H=11008, D=4096)
TOKEN_SPARSITY = 0.2
SPARSITY = 0.4

Z80 = 0.8416212335729143
ZLO = Z80 - 0.065
ZHI = Z80 + 0.080
N_BISECT = 10        # bisection iterations (bracket -> gap <= 8)
N_BISECT_ACT = 8     # of tile-1's iterations, how many run on ScalarE (Sign)


def dims_for(S, H, D):
    assert H % 128 == 0 and H % 16 == 0 and D % N_CORES == 0
    d = {}
    d["S"], d["H"], d["D"] = S, H, D
    d["SLOC"] = S // N_CORES
    assert d["SLOC"] % 128 == 0
    d["NTT"] = d["SLOC"] // 128
    d["DLOC"] = D // N_CORES
    d["KTOK"] = int(H * TOKEN_SPARSITY)
    d["NCORE"] = int(H * SPARSITY)
    d["CH"] = H // 128
    d["NCP"] = ((d["NCORE"] + 127) // 128) * 128
    d["KT"] = d["NCP"] // 128
    d["HP"] = H + 128
    d["YF"] = H // 16
    d["NPAD"] = d["NCP"] - d["NCORE"]
    d["YP"] = (d["NPAD"] + 15) // 16
    assert 16 * d["YP"] <= 128
    d["CBITS"] = max(1, int(np.ceil(np.log2(S + 1))))
    d["JBITS"] = max(1, int(np.ceil(np.log2(H + 16 * d["YP"] + 1))))
    return d


def build_program(S=REAL["S"], H=REAL["H"], D=REAL["D"]):
    d = dims_for(S, H, D)
    SLOC, NTT, DLOC = d["SLOC"], d["NTT"], d["DLOC"]
    KTOK, NCORE, CH = d["KTOK"], d["NCORE"], d["CH"]
    NCP, KT, YF, NPAD, YP = d["NCP"], d["KT"], d["YF"], d["NPAD"], d["YP"]
    HP = d["HP"]
    CBITS, JBITS = d["CBITS"], d["JBITS"]
    JBIG = float(2 ** JBITS)

    nc = bass.Bass("TRN2", num_devices=N_CORES)

    xs_d = nc.dram_tensor("xs", [SLOC, H], F32, kind="ExternalInput")
    xt_d = nc.dram_tensor("xt", [HP, S], F16, kind="ExternalInput")
    wt_d = nc.dram_tensor("wt", [HP, DLOC], F16, kind="ExternalInput")
    out_d = nc.dram_tensor("out", [S, DLOC], F32, kind="ExternalOutput")
    cc_in = nc.dram_tensor("cc_in", [128, CH], F32)
    cc_out = nc.dram_tensor("cc_out", [128, CH], F32, addr_space="Shared")

    with TileContext(nc) as tc:
        with tc.tile_pool(name="state", bufs=1) as st:
            ones16 = st.tile([128, 1], F16)
            nc.vector.memset(ones16[:], 1.0)
            ones32 = st.tile([128, 1], F32)
            nc.vector.memset(ones32[:], 1.0)
            onesrow = st.tile([1, 128], F32)
            nc.vector.memset(onesrow[:], 1.0)
            io8 = st.tile([128, 8], F32)
            i_io8 = nc.gpsimd.iota(io8[:], pattern=[[1, 8]], base=0,
                                   channel_multiplier=0,
                                   allow_small_or_imprecise_dtypes=True)
            compR = st.tile([128, NCP // 16], I16, tag="compR")
            iota_insts = [i_io8]

            with tc.tile_pool(name="cnt", bufs=1) as cp, \
                 tc.tile_pool(name="psc", bufs=1, space="PSUM") as psc, \
                 tc.tile_pool(name="pss", bufs=1, space="PSUM") as pss:

                # ---------- phase A: per-token thresholds, sel, counts --------
                xs_t = [cp.tile([128, H], F32, tag=f"xs{t}", name=f"xs_t{t}") for t in range(NTT)]
                scr = cp.tile([128, H], U8, tag="scr")
                psum_cnt = psc.tile([128, CH], F32)
                for t in range(NTT):
                    nc.sync.dma_start(xs_t[t][:], xs_d[t * 128:(t + 1) * 128, :])

                A_t, B_t, CB_t, TS_t = [], [], [], []
                for t in range(NTT):
                    A_t.append(st.tile([128, 1], F32, tag=f"A{t}", name=f"A{t}"))
                    B_t.append(st.tile([128, 1], F32, tag=f"B{t}", name=f"B{t}"))
                    CB_t.append(st.tile([128, 1], F32, tag=f"CB{t}", name=f"CB{t}"))
                    TS_t.append(st.tile([128, 1], F32, tag=f"TS{t}", name=f"TS{t}"))

                for t in range(NTT):
                    x = xs_t[t]
                    s1 = st.tile([128, 1], F32, tag=f"s1{t}")
                    s2 = st.tile([128, 1], F32, tag=f"s2{t}")
                    stscr = cp.tile([128, H], F16, tag="bigscr")
                    nc.scalar.activation(stscr[:], x[:], AF.Copy, accum_out=s1[:])
                    stscr2 = cp.tile([128, H], F16, tag="bigscr")
                    nc.scalar.activation(stscr2[:], x[:], AF.Square, 0.0, 1.0, 0.0,
                                         accum_out=s2[:])
                    mu = st.tile([128, 1], F32, tag=f"mu{t}")
                    var = st.tile([128, 1], F32, tag=f"var{t}")
                    sig = st.tile([128, 1], F32, tag=f"sig{t}")
                    musq = st.tile([128, 1], F32, tag=f"musq{t}")
                    nc.vector.tensor_scalar_mul(mu[:], s1[:], 1.0 / H)
                    nc.vector.tensor_scalar_mul(var[:], s2[:], 1.0 / H)
                    nc.vector.tensor_tensor(out=musq[:], in0=mu[:], in1=mu[:],
                                            op=OP.mult)
                    nc.vector.tensor_tensor(out=var[:], in0=var[:], in1=musq[:],
                                            op=OP.subtract)
                    nc.scalar.sqrt(sig[:], var[:])
                    nc.vector.scalar_tensor_tensor(A_t[t][:], sig[:], ZLO, mu[:],
                                                   op0=OP.mult, op1=OP.add)
                    nc.vector.scalar_tensor_tensor(B_t[t][:], sig[:], ZHI, mu[:],
                                                   op0=OP.mult, op1=OP.add)
                    nc.vector.memset(CB_t[t][:], 0.0)

                def probe_dve(t, thr_ap, cout_ap):
                    nc.vector.tensor_scalar(scr[:], xs_t[t][:], thr_ap, None,
                                            op0=OP.is_gt, op1=OP.add,
                                            accum_out=cout_ap)

                def probe_act(t, thr_ap, cout_ap):
                    nthr = st.tile([128, 1], F32, tag="nthr")
                    nc.vector.tensor_scalar_mul(nthr[:], thr_ap, -1.0)
                    acc = st.tile([128, 1], F32, tag="acc")
                    ascr = cp.tile([128, H], F16, tag="bigscr")
                    nc.scalar.activation(ascr[:], xs_t[t][:], AF.Sign, bias=nthr[:],
                                         scale=1.0, accum_out=acc[:])
                    nc.vector.tensor_scalar(cout_ap, acc[:], float(H), 0.5,
                                            op0=OP.add, op1=OP.mult)

                c_pr = [st.tile([128, 1], F32, tag=f"cpr{t}", name=f"cpr{t}") for t in range(NTT)]
                tmid = [st.tile([128, 1], F32, tag=f"tmid{t}", name=f"tmid{t}") for t in range(NTT)]
                mge = st.tile([128, 1], U8, tag="mge")
                mlt = st.tile([128, 1], U8, tag="mlt")

                def bis_update(t, c_ap, mid_ap):
                    nc.vector.tensor_scalar(mge[:], c_ap, float(KTOK), None,
                                            op0=OP.is_ge)
                    nc.vector.copy_predicated(A_t[t][:], mge[:], mid_ap)
                    nc.vector.tensor_scalar(mlt[:], c_ap, float(KTOK), None,
                                            op0=OP.is_lt)
                    nc.vector.copy_predicated(B_t[t][:], mlt[:], mid_ap)
                    nc.vector.copy_predicated(CB_t[t][:], mlt[:], c_ap)

                for it in range(N_BISECT):
                    for t in range(NTT):
                        nc.vector.tensor_tensor(out=tmid[t][:], in0=A_t[t][:],
                                                in1=B_t[t][:], op=OP.add)
                        nc.vector.tensor_scalar_mul(tmid[t][:], tmid[t][:], 0.5)
                        if t % 2 == 1 and it < N_BISECT_ACT:
                            probe_act(t, tmid[t][:], c_pr[t][:])
                        else:
                            probe_dve(t, tmid[t][:], c_pr[t][:])
                        bis_update(t, c_pr[t][:], tmid[t][:])

                # finisher: t* = (KTOK - CB)-th largest among values <= B
                for t in range(NTT):
                    yband = cp.tile([128, H], F32, tag="yband")
                    nc.vector.scalar_tensor_tensor(yband[:], xs_t[t][:], B_t[t][:],
                                                   xs_t[t][:], op0=OP.is_le,
                                                   op1=OP.mult)
                    m8 = st.tile([128, 8], F32, tag=f"m8{t}")
                    nc.vector.max(out=m8[:], in_=yband[:])
                    rm1 = st.tile([128, 1], F32, tag=f"rm1{t}")
                    nc.vector.tensor_scalar(rm1[:], CB_t[t][:], float(-(KTOK - 1)),
                                            -1.0, op0=OP.add, op1=OP.mult)
                    rm1p = st.tile([128, 1], F32, tag=f"rm1p{t}")
                    nc.vector.tensor_scalar(rm1p[:], rm1[:], 1.0, None, op0=OP.add)
                    # windowed rank match (robust to a +-0.5 CB offset from the
                    # ScalarE sign-count path): pick i = ceil(rm1)
                    sel8 = st.tile([128, 8], F32, tag=f"sel8{t}")
                    nc.vector.scalar_tensor_tensor(sel8[:], io8[:], rm1[:], m8[:],
                                                   op0=OP.is_ge, op1=OP.mult)
                    sel8b = st.tile([128, 8], F32, tag=f"sel8b{t}")
                    nc.vector.scalar_tensor_tensor(sel8b[:], io8[:], rm1p[:],
                                                   sel8[:], op0=OP.is_lt,
                                                   op1=OP.mult,
                                                   accum_out=TS_t[t][:])

                counts2 = cp.tile([128, CH], F32, tag="counts2")
                for t in range(NTT):
                    sel = cp.tile([128, H], F16, tag="sel", name=f"sel{t}")
                    nc.vector.tensor_scalar(sel[:], xs_t[t][:], TS_t[t][:], None,
                                            op0=OP.is_ge)
                    for f in range(CH):
                        nc.tensor.matmul(psum_cnt[:, f:f + 1], sel[:, f::CH],
                                         ones16[:], start=True, stop=True)
                    if t == 0:
                        nc.vector.tensor_copy(counts2[:], psum_cnt[:])
                    else:
                        nc.vector.tensor_tensor(out=counts2[:], in0=counts2[:],
                                                in1=psum_cnt[:], op=OP.add)
                nc.sync.dma_start(cc_in[:], counts2[:])
                nc.gpsimd.collective_compute(
                    "AllReduce", OP.add,
                    replica_groups=[[i for i in range(N_CORES)]],
                    ins=[cc_in[:].opt()], outs=[cc_out[:].opt()],
                )

                # ---------- phase B: tau + J* ---------------------------------
                call = cp.tile([128, CH], F32, tag="call")
                nc.sync.dma_start(call[:], cc_out[:])
                jt = cp.tile([128, CH], F32, tag="jt")
                i_jt = nc.gpsimd.iota(jt[:], pattern=[[1, CH]], base=0,
                                      channel_multiplier=CH,
                                      allow_small_or_imprecise_dtypes=True)
                iota_insts.append(i_jt)
                jmB = cp.tile([128, CH], F32, tag="jmB")
                nc.vector.tensor_scalar(jmB[:], jt[:], -JBIG, None, op0=OP.add)

                scr86 = cp.tile([128, CH], U8, tag="scr86")
                gpart = st.tile([128, 1], F32, tag="gpart")
                Gb = st.tile([128, 1], F32, tag="Gb")
                g1 = st.tile([1, 1], F32, tag="g1")

                def total_count(src_ap, thr_ap, op):
                    nc.vector.tensor_scalar(scr86[:], src_ap, thr_ap, None,
                                            op0=op, op1=OP.add, accum_out=gpart[:])
                    p1 = pss.tile([1, 1], F32, tag="p1")
                    nc.tensor.matmul(p1[:], gpart[:], ones32[:], start=True,
                                     stop=True)
                    nc.vector.tensor_copy(g1[:], p1[:])
                    p2 = pss.tile([128, 1], F32, tag="p2")
                    nc.tensor.matmul(p2[:], onesrow[:], g1[:], start=True, stop=True)
                    nc.vector.tensor_copy(Gb[:], p2[:])

                lo = st.tile([128, 1], F32, tag="lo")
                hi = st.tile([128, 1], F32, tag="hi")
                Ghi = st.tile([128, 1], F32, tag="Ghi")
                mid = st.tile([128, 1], F32, tag="mid")
                nc.vector.memset(lo[:], -0.5)
                nc.vector.memset(hi[:], 2.0 ** CBITS - 0.5)
                nc.vector.memset(Ghi[:], 0.0)
                for it in range(CBITS):
                    nc.vector.tensor_tensor(out=mid[:], in0=lo[:], in1=hi[:],
                                            op=OP.add)
                    nc.vector.tensor_scalar_mul(mid[:], mid[:], 0.5)
                    total_count(call[:], mid[:], OP.is_gt)
                    nc.vector.tensor_scalar(mge[:], Gb[:], float(NCORE), None,
                                            op0=OP.is_ge)
                    nc.vector.copy_predicated(lo[:], mge[:], mid[:])
                    nc.vector.tensor_scalar(mlt[:], Gb[:], float(NCORE), None,
                                            op0=OP.is_lt)
                    nc.vector.copy_predicated(hi[:], mlt[:], mid[:])
                    nc.vector.copy_predicated(Ghi[:], mlt[:], Gb[:])
                tau = st.tile([128, 1], F32, tag="tau")
                nc.vector.tensor_scalar(tau[:], lo[:], 0.5, None, op0=OP.add)
                rr = st.tile([128, 1], F32, tag="rr")
                nc.vector.tensor_scalar(rr[:], Ghi[:], float(-NCORE), -1.0,
                                        op0=OP.add, op1=OP.mult)

                mj = cp.tile([128, CH], F32, tag="mj")
                nc.vector.scalar_tensor_tensor(mj[:], call[:], tau[:], jmB[:],
                                               op0=OP.is_equal, op1=OP.mult)
                nc.vector.tensor_scalar(mj[:], mj[:], JBIG, None, op0=OP.add)

                jlo = st.tile([128, 1], F32, tag="jlo")
                jhi = st.tile([128, 1], F32, tag="jhi")
                nc.vector.memset(jlo[:], -0.5)
                nc.vector.memset(jhi[:], 2.0 ** JBITS - 0.5)
                for it in range(JBITS):
                    nc.vector.tensor_tensor(out=mid[:], in0=jlo[:], in1=jhi[:],
                                            op=OP.add)
                    nc.vector.tensor_scalar_mul(mid[:], mid[:], 0.5)
                    total_count(mj[:], mid[:], OP.is_le)
                    nc.vector.tensor_tensor(out=mge[:], in0=Gb[:], in1=rr[:],
                                            op=OP.is_ge)
                    nc.vector.copy_predicated(jhi[:], mge[:], mid[:])
                    nc.vector.tensor_tensor(out=mlt[:], in0=Gb[:], in1=rr[:],
                                            op=OP.is_lt)
                    nc.vector.copy_predicated(jlo[:], mlt[:], mid[:])
                jstar = st.tile([128, 1], F32, tag="jstar")
                nc.vector.tensor_scalar(jstar[:], jlo[:], 0.5, None, op0=OP.add)

                # ---------- phase C: y build + sparse_gather ------------------
                ycnt = cp.tile([16, YF], F32, tag="ycnt")
                nc.sync.dma_start(ycnt[:],
                                  cc_out[:].rearrange("(a b) c -> a (b c)", a=16))
                jy = cp.tile([16, YF], F32, tag="jy")
                i_jy = nc.gpsimd.iota(jy[:], pattern=[[1, YF]], base=0,
                                      channel_multiplier=YF,
                                      allow_small_or_imprecise_dtypes=True)
                iota_insts.append(i_jy)
                y = cp.tile([16, YF + YP], F32, tag="y")
                c1y = cp.tile([16, YF], F32, tag="c1y")
                nc.vector.tensor_scalar(c1y[:], ycnt[:], tau[:16, :], None,
                                        op0=OP.is_gt)
                jmBy = cp.tile([16, YF], F32, tag="ytmp")
                nc.vector.tensor_scalar(jmBy[:], jy[:], -JBIG, None, op0=OP.add)
                mjy = cp.tile([16, YF], F32, tag="mjy")
                nc.vector.scalar_tensor_tensor(mjy[:], ycnt[:], tau[:16, :], jmBy[:],
                                               op0=OP.is_equal, op1=OP.mult)
                nc.vector.tensor_scalar(mjy[:], mjy[:], JBIG, None, op0=OP.add)
                c2y = cp.tile([16, YF], F32, tag="ytmp")
                nc.vector.tensor_scalar(c2y[:], mjy[:], jstar[:16, :], None,
                                        op0=OP.is_le)
                nc.vector.tensor_tensor(out=c1y[:], in0=c1y[:], in1=c2y[:],
                                        op=OP.add)
                jy1 = cp.tile([16, YF], F32, tag="ytmp")
                nc.vector.tensor_scalar(jy1[:], jy[:], 1.0, None, op0=OP.add)
                nc.vector.tensor_tensor(out=y[:, :YF], in0=c1y[:], in1=jy1[:],
                                        op=OP.mult)
                nc.vector.tensor_scalar(y[:, :YF], y[:, :YF], -1.0, None,
                                        op0=OP.add)
                pv = cp.tile([16, YP], F32, tag="pv")
                i_pv = nc.gpsimd.iota(pv[:], pattern=[[1, YP]], base=H,
                                      channel_multiplier=YP,
                                      allow_small_or_imprecise_dtypes=True)
                iota_insts.append(i_pv)
                pm = cp.tile([16, YP], F32, tag="pm")
                nc.vector.tensor_scalar(pm[:], pv[:], float(H + NPAD - 1), None,
                                        op0=OP.is_le)
                pv1 = cp.tile([16, YP], F32, tag="pv1")
                nc.vector.tensor_scalar(pv1[:], pv[:], 1.0, None, op0=OP.add)
                nc.vector.tensor_tensor(out=y[:, YF:], in0=pm[:], in1=pv1[:],
                                        op=OP.mult)
                nc.vector.tensor_scalar(y[:, YF:], y[:, YF:], -1.0, None,
                                        op0=OP.add)

                comp = cp.tile([16, NCP // 16], F32, tag="comp")
                nfound = st.tile([1, 1], U32, tag="nfound")
                i_lib8 = nc.gpsimd.load_library(library_config.sparse_gather)
                for dep in iota_insts:
                    add_dep_helper(i_lib8.ins, dep.ins, sync=False,
                                   reason="lib order")
                i_sg = nc.gpsimd.sparse_gather(comp[:], y[:], num_found=nfound[:])
                add_dep_helper(i_sg.ins, i_lib8.ins, sync=False, reason="lib order")

                comp16 = cp.tile([16, NCP // 16], I16, tag="comp16")
                nc.vector.tensor_copy(comp16[:], comp[:])
                for r in range(8):
                    nc.sync.dma_start(compR[16 * r:16 * r + 16, :], comp16[:])

            # ---------- phase D: gathers + reduced GEMM -----------------------
            i_lib3 = nc.gpsimd.load_library(library_config.mlp)
            add_dep_helper(i_lib3.ins, i_sg.ins, sync=False, reason="lib order")

            with tc.tile_pool(name="gemm", bufs=1) as gp, \
                 tc.tile_pool(name="outp", bufs=3) as op_, \
                 tc.tile_pool(name="pso", bufs=1, space="PSUM") as pso:
                xtc = [gp.tile([128, 1, S], F16, tag=f"xtc{kt}", name=f"xtc{kt}") for kt in range(KT)]
                wtc = [gp.tile([128, 1, DLOC], F16, tag=f"wtc{kt}", name=f"wtc{kt}")
                       for kt in range(KT)]
                prev = i_lib3
                n128_reg = nc.gpsimd.to_reg(128)
                for kt in range(KT):
                    ix = compR[:, 8 * kt:8 * kt + 8]
                    gx = nc.gpsimd.dma_gather(xtc[kt][:], xt_d[:], ix, num_idxs=128,
                                              num_idxs_reg=n128_reg, elem_size=S)
                    add_dep_helper(gx.ins, prev.ins, sync=False, reason="lib order")
                    gw = nc.gpsimd.dma_gather(wtc[kt][:], wt_d[:], ix, num_idxs=128,
                                              num_idxs_reg=n128_reg, elem_size=DLOC)
                    add_dep_helper(gw.ins, gx.ins, sync=False, reason="lib order")
                    prev = gw

                MT = S // 128
                MB = 8
                for mb in range(0, MT, MB):
                    nmb = min(MB, MT - mb)
                    ptiles = [pso.tile([128, DLOC], F32, tag=f"po{i}", name=f"po{mb}_{i}")
                              for i in range(nmb)]
                    for kt in range(KT):
                        for i in range(nmb):
                            m = mb + i
                            nc.tensor.matmul(
                                ptiles[i][:],
                                xtc[kt][:, 0, 128 * m:128 * (m + 1)],
                                wtc[kt][:, 0, :],
                                start=(kt == 0), stop=(kt == KT - 1))
                    for i in range(nmb):
                        m = mb + i
                        outs = op_.tile([128, DLOC], F32, tag="outs")
                        if i % 2 == 0:
                            nc.vector.tensor_copy(outs[:], ptiles[i][:])
                        else:
                            nc.scalar.copy(outs[:], ptiles[i][:])
                        nc.sync.dma_start(out_d[128 * m:128 * (m + 1), :], outs[:])

    return nc, d


def _split_excess_waits(nc):
    """This walrus build rejects >1 sync wait on several instruction structs;
    hoist extra waits into single-wait NOPs placed just before, same engine."""
    for f in nc.m.functions:
        for bb in f.blocks:
            newi = []
            changed = False
            for ins in bb.instructions:
                si = ins.sync_info
                maxw = 1
                if si is not None and len(si.on_wait) > maxw:
                    waits = list(si.on_wait)
                    keep = waits[-maxw:]
                    for i, w in enumerate(waits[:-maxw]):
                        nop = mybir.InstNoOp(name=f"{ins.name}-ws{i}")
                        nop.engine = ins.engine
                        nop.sync_info = mybir.SyncInfo(on_wait=[w], on_update=[])
                        newi.append(nop)
                    ins.sync_info = mybir.SyncInfo(
                        on_wait=list(keep), on_update=list(si.on_update))
                    changed = True
                newi.append(ins)
            if changed:
                bb.instructions[:] = newi


_CACHE = {}


def _get_program():
    if "real" not in _CACHE:
        nc, d = build_program()
        # populate .instr bytes for extended gpsimd instructions
        # (sparse_gather, dma_gather, library reload) - raw Bass doesn't
        # run this codegen pass and walrus errors "ISA wrong length" without it
        from concourse.library_overlay import lower_extended_insts
        lower_extended_insts(nc)
        _split_excess_waits(nc)
        _CACHE["real"] = (nc, d)
    return _CACHE["real"]


def make_in_maps(x2d, W, d):
    """Host-side prep: f32 token slices, padded transposed f16 x and W shards."""
    H, S = d["H"], d["S"]
    HP, SLOC, DLOC = d["HP"], d["SLOC"], d["DLOC"]
    xt = np.zeros((HP, S), np.float16)
    xt[:H, :] = x2d.T.astype(np.float16)
    in_maps = []
    for c in range(N_CORES):
        wt = np.zeros((HP, DLOC), np.float16)
        wt[:H, :] = W[c * DLOC:(c + 1) * DLOC, :].T.astype(np.float16)
        in_maps.append({
            "xs": np.ascontiguousarray(x2d[c * SLOC:(c + 1) * SLOC, :]),
            "xt": xt,
            "wt": wt,
        })
    return in_maps


def kernel(x, W):
    x = np.asarray(x)
    W = np.asarray(W)
    B, S, H = x.shape
    D = W.shape[0]
    assert (S, H, D) == (REAL["S"], REAL["H"], REAL["D"])
    nc, d = _get_program()
    in_maps = make_in_maps(x.reshape(S, H), W, d)
    res = run_bass_kernel_spmd(nc, in_maps, core_ids=list(range(N_CORES)))
    out = np.concatenate([res.results[c]["out"] for c in range(N_CORES)], axis=1)
    return out.reshape(B, S, D).astype(np.float32)



# revision 15
# speedup vs baseline: 1.1572x; 1.1572x over previous
"""Trainium2 Bass kernel for nn_CustomMLPLayer_74526272520565 (topk_masking).

Reference semantics:
  core_idx = top-n_core neurons by how often they appear in each token's
             top-k_tok activations (count ties broken toward lower index)
  out = x[..., core_idx] @ W[:, core_idx].T

Distribution (8 NeuronCores): token-sharded counting (AllReduced), then
2D-sharded GEMM (2 token-halves x 4 col-blocks of W rows).

Per-core device algorithm:
  A. For its 256-token slice: exact k_tok-th largest activation per token.
     Bisection probes run on a host-provided f16 copy (DVE 4x mode / ScalarE
     Sign), bracketed by f16 stats; the finisher extracts the top-8 f32
     values of the f16-band and rank-selects the exact f32 threshold.
     sel = (x32 >= t*); counts[j] = sum_s sel[s, j] via PE matmuls.
  B. AllReduce counts; exact core-set threshold tau via integer bisection
     (DVE compare + gpsimd partition_all_reduce); tie selection among
     count==tau via prefix-scan rank (tensor_tensor_scan + triangular
     matmul), no second bisection.
  C. Compact the 4403 core indices (gpsimd sparse_gather) + 77 zero-row pads.
  D. 2D GEMM sharding: dma_gather rows of a host-fused f16 tensor
     [x^T half | W^T block] (one 4KB-row gather per K-tile feeds both
     operands); reduced GEMM (K=4480) accumulated in PSUM f32, two M-half
     sweeps of 8x [128,512] accumulators.
"""
import numpy as np

import concourse.bass as bass
import concourse.bass_isa as bass_isa
import concourse.mybir as mybir
from concourse.tile import TileContext
from concourse.tile_rust import add_dep_helper
from concourse import library_config
from concourse.bass_utils import run_bass_kernel_spmd

AF = mybir.ActivationFunctionType
OP = mybir.AluOpType
F32 = mybir.dt.float32
F16 = mybir.dt.float16
U8 = mybir.dt.uint8
I16 = mybir.dt.int16
U32 = mybir.dt.uint32

N_CORES = 8

REAL = dict(S=2048, H=11008, D=4096)
TOKEN_SPARSITY = 0.2
SPARSITY = 0.4

Z80 = 0.8416212335729143
ZLO = Z80 - 0.065
ZHI = Z80 + 0.080
N_BISECT = 10        # bisection iterations (bracket -> gap <= 8)
SC_PROBES_T1 = 7     # of tile-1's iterations, how many run on ScalarE (Sign)


def dims_for(S, H, D):
    assert H % 128 == 0 and H % 16 == 0 and D % N_CORES == 0
    d = {}
    d["S"], d["H"], d["D"] = S, H, D
    d["SLOC"] = S // N_CORES
    assert d["SLOC"] % 128 == 0
    d["NTT"] = d["SLOC"] // 128
    d["DLOC"] = D // N_CORES
    # GEMM 2D sharding: 2 token-halves x 4 col-blocks
    d["SG"] = S // 2
    d["DG"] = D // 4
    assert d["SG"] % 128 == 0 and d["DG"] % 128 == 0
    d["KTOK"] = int(H * TOKEN_SPARSITY)
    d["NCORE"] = int(H * SPARSITY)
    d["CH"] = H // 128
    d["NCP"] = ((d["NCORE"] + 127) // 128) * 128
    d["KT"] = d["NCP"] // 128
    d["HP"] = H + 128
    d["YF"] = H // 16
    d["NPAD"] = d["NCP"] - d["NCORE"]
    d["YP"] = (d["NPAD"] + 15) // 16
    assert 16 * d["YP"] <= 128
    d["CBITS"] = max(1, int(np.ceil(np.log2(S + 1))))
    return d


def build_program(S=REAL["S"], H=REAL["H"], D=REAL["D"]):
    d = dims_for(S, H, D)
    SLOC, NTT = d["SLOC"], d["NTT"]
    KTOK, NCORE, CH = d["KTOK"], d["NCORE"], d["CH"]
    NCP, KT, YF, NPAD, YP = d["NCP"], d["KT"], d["YF"], d["NPAD"], d["YP"]
    HP = d["HP"]
    CBITS = d["CBITS"]
    SG, DG = d["SG"], d["DG"]

    nc = bass.Bass("TRN2", num_devices=N_CORES)

    xs_d = nc.dram_tensor("xs", [SLOC, H], F32, kind="ExternalInput")
    xs16_d = nc.dram_tensor("xs16", [SLOC, H], F16, kind="ExternalInput")
    xw_d = nc.dram_tensor("xw", [HP, 2 * DG], F16, kind="ExternalInput")
    out_d = nc.dram_tensor("out", [SG, DG], F32, kind="ExternalOutput")
    cc_in = nc.dram_tensor("cc_in", [128, CH], F32)
    cc_out = nc.dram_tensor("cc_out", [128, CH], F32, addr_space="Shared")

    with TileContext(nc) as tc:
        with tc.tile_pool(name="state", bufs=1) as st:
            ones16 = st.tile([128, 1], F16)
            nc.vector.memset(ones16[:], 1.0)
            io8 = st.tile([128, 8], F32)
            i_io8 = nc.gpsimd.iota(io8[:], pattern=[[1, 8]], base=0,
                                   channel_multiplier=0,
                                   allow_small_or_imprecise_dtypes=True)
            compR = st.tile([128, NCP // 16], I16, tag="compR")
            # iotas for phase B/C (issued early; all standard-lib gpsimd ops
            # must precede the first library load)
            jy1 = st.tile([16, YF], F32, tag="jy1")
            i_jy = nc.gpsimd.iota(jy1[:], pattern=[[1, YF]], base=1,
                                  channel_multiplier=YF,
                                  allow_small_or_imprecise_dtypes=True)
            pv = st.tile([16, YP], F32, tag="pv")
            i_pv = nc.gpsimd.iota(pv[:], pattern=[[1, YP]], base=H,
                                  channel_multiplier=YP,
                                  allow_small_or_imprecise_dtypes=True)
            # strict upper-triangular [16,16] ones: UT[p, c] = (p < c)
            itr = st.tile([16, 16], F32, tag="itr")
            i_itr = nc.gpsimd.iota(itr[:], pattern=[[0, 16]], base=0,
                                   channel_multiplier=1,
                                   allow_small_or_imprecise_dtypes=True)
            itc = st.tile([16, 16], F32, tag="itc")
            i_itc = nc.gpsimd.iota(itc[:], pattern=[[1, 16]], base=0,
                                   channel_multiplier=0,
                                   allow_small_or_imprecise_dtypes=True)
            ut16 = st.tile([16, 16], F32, tag="ut16")
            nc.vector.tensor_tensor(out=ut16[:], in0=itr[:], in1=itc[:],
                                    op=OP.is_lt)
            iota_insts = [i_io8, i_jy, i_pv, i_itr, i_itc]

            with tc.tile_pool(name="cnt", bufs=1) as cp, \
                 tc.tile_pool(name="psc", bufs=1, space="PSUM") as psc, \
                 tc.tile_pool(name="pss", bufs=1, space="PSUM") as pss:

                # ---------- phase A: per-token thresholds, sel, counts --------
                x16_t = [cp.tile([128, H], F16, tag=f"x16{t}", name=f"x16_t{t}")
                         for t in range(NTT)]
                x32b = cp.tile([128, H], F32, tag="x32b", name="x32b0")
                scr16 = cp.tile([128, H], F16, tag="scr16")
                sscr16 = cp.tile([128, H], F16, tag="sscr16")
                psum_cnt = psc.tile([128, CH], F32)
                for t in range(NTT):
                    nc.sync.dma_start(x16_t[t][:],
                                      xs16_d[t * 128:(t + 1) * 128, :])
                nc.sync.dma_start(x32b[:], xs_d[0:128, :])

                A_t, B_t, CB_t, TS_t = [], [], [], []
                for t in range(NTT):
                    A_t.append(st.tile([128, 1], F32, tag=f"A{t}", name=f"A{t}"))
                    B_t.append(st.tile([128, 1], F32, tag=f"B{t}", name=f"B{t}"))
                    CB_t.append(st.tile([128, 1], F32, tag=f"CB{t}",
                                        name=f"CB{t}"))
                    TS_t.append(st.tile([128, 1], F32, tag=f"TS{t}",
                                        name=f"TS{t}"))

                # stats: s1 on DVE (f16 copy-accum), s2 on ScalarE (square)
                for t in range(NTT):
                    s1 = st.tile([128, 1], F32, tag=f"s1{t}")
                    s2 = st.tile([128, 1], F32, tag=f"s2{t}")
                    nc.vector.tensor_scalar(scr16[:], x16_t[t][:], 0.0, None,
                                            op0=OP.add, op1=OP.add,
                                            accum_out=s1[:])
                    nc.scalar.activation(sscr16[:], x16_t[t][:], AF.Square,
                                         0.0, 1.0, 0.0, accum_out=s2[:])
                    mu = st.tile([128, 1], F32, tag=f"mu{t}")
                    var = st.tile([128, 1], F32, tag=f"var{t}")
                    sig = st.tile([128, 1], F32, tag=f"sig{t}")
                    musq = st.tile([128, 1], F32, tag=f"musq{t}")
                    nc.vector.tensor_scalar_mul(mu[:], s1[:], 1.0 / H)
                    nc.vector.tensor_scalar_mul(var[:], s2[:], 1.0 / H)
                    nc.vector.tensor_tensor(out=musq[:], in0=mu[:], in1=mu[:],
                                            op=OP.mult)
                    nc.vector.tensor_tensor(out=var[:], in0=var[:],
                                            in1=musq[:], op=OP.subtract)
                    nc.scalar.sqrt(sig[:], var[:])
                    nc.vector.scalar_tensor_tensor(A_t[t][:], sig[:], ZLO,
                                                   mu[:], op0=OP.mult,
                                                   op1=OP.add)
                    nc.vector.scalar_tensor_tensor(B_t[t][:], sig[:], ZHI,
                                                   mu[:], op0=OP.mult,
                                                   op1=OP.add)
                    nc.vector.memset(CB_t[t][:], 0.0)

                def probe_dve16(t, thr_ap, cout_ap):
                    nc.vector.tensor_scalar(scr16[:], x16_t[t][:], thr_ap,
                                            None, op0=OP.is_gt, op1=OP.add,
                                            accum_out=cout_ap)

                def probe_act16(t, thr_ap, cout_ap):
                    nthr = st.tile([128, 1], F32, tag="nthr")
                    nc.vector.tensor_scalar_mul(nthr[:], thr_ap, -1.0)
                    acc = st.tile([128, 1], F32, tag="acc")
                    nc.scalar.activation(sscr16[:], x16_t[t][:], AF.Sign,
                                         bias=nthr[:], scale=1.0,
                                         accum_out=acc[:])
                    nc.vector.tensor_scalar(cout_ap, acc[:], float(H), 0.5,
                                            op0=OP.add, op1=OP.mult)

                c_pr = [st.tile([128, 1], F32, tag=f"cpr{t}", name=f"cpr{t}")
                        for t in range(NTT)]
                tmid = [st.tile([128, 1], F32, tag=f"tmid{t}", name=f"tmid{t}")
                        for t in range(NTT)]
                mge = st.tile([128, 1], U8, tag="mge")
                mlt = st.tile([128, 1], U8, tag="mlt")

                def bis_update(t, c_ap, mid_ap):
                    nc.vector.tensor_scalar(mge[:], c_ap, float(KTOK), None,
                                            op0=OP.is_ge)
                    nc.vector.copy_predicated(A_t[t][:], mge[:], mid_ap)
                    nc.vector.tensor_scalar(mlt[:], c_ap, float(KTOK), None,
                                            op0=OP.is_lt)
                    nc.vector.copy_predicated(B_t[t][:], mlt[:], mid_ap)
                    nc.vector.copy_predicated(CB_t[t][:], mlt[:], c_ap)

                for it in range(N_BISECT):
                    for t in range(NTT):
                        nc.vector.tensor_tensor(out=tmid[t][:], in0=A_t[t][:],
                                                in1=B_t[t][:], op=OP.add)
                        nc.vector.tensor_scalar_mul(tmid[t][:], tmid[t][:],
                                                    0.5)
                        if t % 2 == 1 and it < SC_PROBES_T1:
                            probe_act16(t, tmid[t][:], c_pr[t][:])
                        else:
                            probe_dve16(t, tmid[t][:], c_pr[t][:])
                        bis_update(t, c_pr[t][:], tmid[t][:])

                # finisher: t* = (KTOK - CB)-th largest f32 among {x16 <= B}
                counts2 = cp.tile([128, CH], F32, tag="counts2")
                yband = cp.tile([128, H], F32, tag="yband")
                for t in range(NTT):
                    if t > 0:
                        nc.sync.dma_start(x32b[:],
                                          xs_d[t * 128:(t + 1) * 128, :])
                    nc.vector.scalar_tensor_tensor(yband[:], x16_t[t][:],
                                                   B_t[t][:], x32b[:],
                                                   op0=OP.is_le, op1=OP.mult)
                    m8 = st.tile([128, 8], F32, tag=f"m8{t}")
                    nc.vector.max(out=m8[:], in_=yband[:])
                    rm1 = st.tile([128, 1], F32, tag=f"rm1{t}")
                    nc.vector.tensor_scalar(rm1[:], CB_t[t][:],
                                            float(-(KTOK - 1)), -1.0,
                                            op0=OP.add, op1=OP.mult)
                    rm1p = st.tile([128, 1], F32, tag=f"rm1p{t}")
                    nc.vector.tensor_scalar(rm1p[:], rm1[:], 1.0, None,
                                            op0=OP.add)
                    # windowed rank match (robust to +-0.5 CB offset from the
                    # ScalarE sign-count path): pick i = ceil(rm1)
                    sel8 = st.tile([128, 8], F32, tag=f"sel8{t}")
                    nc.vector.scalar_tensor_tensor(sel8[:], io8[:], rm1[:],
                                                   m8[:], op0=OP.is_ge,
                                                   op1=OP.mult)
                    sel8b = st.tile([128, 8], F32, tag=f"sel8b{t}")
                    nc.vector.scalar_tensor_tensor(sel8b[:], io8[:], rm1p[:],
                                                   sel8[:], op0=OP.is_lt,
                                                   op1=OP.mult,
                                                   accum_out=TS_t[t][:])
                    # sel reuses x16_t[t]'s buffer (x16 dead after yband)
                    sel = cp.tile([128, H], F16, tag=f"x16{t}",
                                  name=f"sel{t}")
                    nc.vector.tensor_scalar(sel[:], x32b[:], TS_t[t][:], None,
                                            op0=OP.is_ge)
                    for f in range(CH):
                        nc.tensor.matmul(psum_cnt[:, f:f + 1], sel[:, f::CH],
                                         ones16[:], start=True, stop=True)
                    if t == 0:
                        nc.vector.tensor_copy(counts2[:], psum_cnt[:])
                    else:
                        nc.vector.tensor_tensor(out=counts2[:], in0=counts2[:],
                                                in1=psum_cnt[:], op=OP.add)
                nc.sync.dma_start(cc_in[:], counts2[:])
                i_cc = nc.gpsimd.collective_compute(
                    "AllReduce", OP.add,
                    replica_groups=[[i for i in range(N_CORES)]],
                    ins=[cc_in[:].opt()], outs=[cc_out[:].opt()],
                )

                # mlp library (partition_all_reduce) for phase B; load
                # overlaps the AllReduce wait
                i_lib_m1 = nc.gpsimd.load_library(library_config.mlp)
                for dep in iota_insts:
                    add_dep_helper(i_lib_m1.ins, dep.ins, sync=False,
                                   reason="lib order")
                add_dep_helper(i_lib_m1.ins, i_cc.ins, sync=False,
                               reason="lib order")

                # ---------- phase B: tau bisection + prefix-scan ties ---------
                call = cp.tile([128, CH], F32, tag="call")
                nc.sync.dma_start(call[:], cc_out[:])
                ycnt = cp.tile([16, YF], F32, tag="ycnt")
                nc.sync.dma_start(ycnt[:],
                                  cc_out[:].rearrange("(a b) c -> a (b c)",
                                                      a=16))

                scr86 = cp.tile([128, CH], U8, tag="scr86")
                gpart = st.tile([128, 1], F32, tag="gpart")
                Gb = st.tile([128, 1], F32, tag="Gb")

                lo = st.tile([128, 1], F32, tag="lo")
                hi = st.tile([128, 1], F32, tag="hi")
                Ghi = st.tile([128, 1], F32, tag="Ghi")
                mid = st.tile([128, 1], F32, tag="mid")
                nc.vector.memset(lo[:], -0.5)
                nc.vector.memset(hi[:], 2.0 ** CBITS - 0.5)
                nc.vector.memset(Ghi[:], 0.0)
                last_par = i_lib_m1
                for it in range(CBITS):
                    nc.vector.tensor_tensor(out=mid[:], in0=lo[:], in1=hi[:],
                                            op=OP.add)
                    nc.vector.tensor_scalar_mul(mid[:], mid[:], 0.5)
                    nc.vector.tensor_scalar(scr86[:], call[:], mid[:], None,
                                            op0=OP.is_gt, op1=OP.add,
                                            accum_out=gpart[:])
                    par = nc.gpsimd.partition_all_reduce(
                        Gb[:], gpart[:], channels=128,
                        reduce_op=bass_isa.ReduceOp.add)
                    add_dep_helper(par.ins, last_par.ins, sync=False,
                                   reason="lib order")
                    last_par = par
                    nc.vector.tensor_scalar(mge[:], Gb[:], float(NCORE), None,
                                            op0=OP.is_ge)
                    nc.vector.copy_predicated(lo[:], mge[:], mid[:])
                    nc.vector.tensor_scalar(mlt[:], Gb[:], float(NCORE), None,
                                            op0=OP.is_lt)
                    nc.vector.copy_predicated(hi[:], mlt[:], mid[:])
                    nc.vector.copy_predicated(Ghi[:], mlt[:], Gb[:])
                tau = st.tile([128, 1], F32, tag="tau")
                nc.vector.tensor_scalar(tau[:], lo[:], 0.5, None, op0=OP.add)
                rr = st.tile([128, 1], F32, tag="rr")
                nc.vector.tensor_scalar(rr[:], Ghi[:], float(-NCORE), -1.0,
                                        op0=OP.add, op1=OP.mult)

                # ---------- phase C: y build (prefix ties) + sparse_gather ----
                y = cp.tile([16, YF + YP], F32, tag="y")
                ties = cp.tile([16, YF], F32, tag="ties")
                nc.vector.tensor_scalar(ties[:], ycnt[:], tau[:16, :], None,
                                        op0=OP.is_equal)
                zz = cp.tile([16, YF], F32, tag="zz")
                nc.vector.memset(zz[:], 0.0)
                pfx = cp.tile([16, YF], F32, tag="pfx")
                nc.vector.tensor_tensor_scan(pfx[:], ties[:], zz[:], 0.0,
                                             op0=OP.add, op1=OP.add)
                # cross-partition exclusive offsets of tie totals
                pso16 = pss.tile([16, 1], F32, tag="pso16")
                nc.tensor.matmul(pso16[:], ut16[:], pfx[:, YF - 1:YF],
                                 start=True, stop=True)
                off = st.tile([16, 1], F32, tag="off")
                nc.vector.tensor_copy(off[:], pso16[:])
                rnk = cp.tile([16, YF], F32, tag="rnk")
                nc.vector.tensor_scalar(rnk[:], pfx[:], off[:], None,
                                        op0=OP.add)
                msel = cp.tile([16, YF], F32, tag="msel")
                nc.vector.scalar_tensor_tensor(msel[:], rnk[:], rr[:16, :],
                                               ties[:], op0=OP.is_le,
                                               op1=OP.mult)
                c1y = cp.tile([16, YF], F32, tag="c1y")
                nc.vector.tensor_scalar(c1y[:], ycnt[:], tau[:16, :], None,
                                        op0=OP.is_gt)
                nc.vector.tensor_tensor(out=c1y[:], in0=c1y[:], in1=msel[:],
                                        op=OP.add)
                nc.vector.tensor_tensor(out=y[:, :YF], in0=c1y[:], in1=jy1[:],
                                        op=OP.mult)
                nc.vector.tensor_scalar(y[:, :YF], y[:, :YF], -1.0, None,
                                        op0=OP.add)
                pm = cp.tile([16, YP], F32, tag="pm")
                nc.vector.tensor_scalar(pm[:], pv[:], float(H + NPAD - 1),
                                        None, op0=OP.is_le)
                pv1 = cp.tile([16, YP], F32, tag="pv1")
                nc.vector.tensor_scalar(pv1[:], pv[:], 1.0, None, op0=OP.add)
                nc.vector.tensor_tensor(out=y[:, YF:], in0=pm[:], in1=pv1[:],
                                        op=OP.mult)
                nc.vector.tensor_scalar(y[:, YF:], y[:, YF:], -1.0, None,
                                        op0=OP.add)

                comp = cp.tile([16, NCP // 16], F32, tag="comp")
                nfound = st.tile([1, 1], U32, tag="nfound")
                i_lib8 = nc.gpsimd.load_library(library_config.sparse_gather)
                add_dep_helper(i_lib8.ins, last_par.ins, sync=False,
                               reason="lib order")
                i_sg = nc.gpsimd.sparse_gather(comp[:], y[:],
                                               num_found=nfound[:])
                add_dep_helper(i_sg.ins, i_lib8.ins, sync=False,
                               reason="lib order")

                comp16 = cp.tile([16, NCP // 16], I16, tag="comp16")
                nc.vector.tensor_copy(comp16[:], comp[:])
                for r in range(8):
                    nc.sync.dma_start(compR[16 * r:16 * r + 16, :], comp16[:])

            # ---------- phase D: fused gathers + reduced GEMM -----------------
            i_lib3 = nc.gpsimd.load_library(library_config.mlp)
            add_dep_helper(i_lib3.ins, i_sg.ins, sync=False, reason="lib order")

            with tc.tile_pool(name="gemm", bufs=1) as gp, \
                 tc.tile_pool(name="outp", bufs=3) as op_, \
                 tc.tile_pool(name="pso", bufs=1, space="PSUM") as pso:
                xw_t = [gp.tile([128, 1, 2 * DG], F16, tag=f"xw{kt}",
                                name=f"xw{kt}") for kt in range(KT)]
                prev = i_lib3
                n128_reg = nc.gpsimd.to_reg(128)
                for kt in range(KT):
                    ix = compR[:, 8 * kt:8 * kt + 8]
                    g = nc.gpsimd.dma_gather(xw_t[kt][:], xw_d[:], ix,
                                             num_idxs=128,
                                             num_idxs_reg=n128_reg,
                                             elem_size=2 * DG)
                    add_dep_helper(g.ins, prev.ins, sync=False,
                                   reason="lib order")
                    prev = g

                MT = SG // 128
                MH = MT // 2
                for half in range(2):
                    ptiles = [pso.tile([128, 512], F32, tag=f"po{i}",
                                       name=f"po{half}_{i}")
                              for i in range(2 * MH)]
                    for kt in range(KT):
                        for i in range(MH):
                            m = half * MH + i
                            for n in range(2):
                                nc.tensor.matmul(
                                    ptiles[2 * i + n][:],
                                    xw_t[kt][:, 0, 128 * m:128 * (m + 1)],
                                    xw_t[kt][:, 0,
                                             DG + 512 * n:DG + 512 * (n + 1)],
                                    start=(kt == 0), stop=(kt == KT - 1))
                    for i in range(MH):
                        m = half * MH + i
                        outs = op_.tile([128, DG], F32, tag="outs")
                        if i % 2 == 0:
                            nc.vector.tensor_copy(outs[:, :512],
                                                  ptiles[2 * i][:])
                            nc.vector.tensor_copy(outs[:, 512:],
                                                  ptiles[2 * i + 1][:])
                        else:
                            nc.scalar.copy(outs[:, :512], ptiles[2 * i][:])
                            nc.scalar.copy(outs[:, 512:], ptiles[2 * i + 1][:])
                        nc.sync.dma_start(out_d[128 * m:128 * (m + 1), :],
                                          outs[:])

    return nc, d


def _split_excess_waits(nc):
    """This walrus build rejects >1 sync wait on several instruction structs;
    hoist extra waits into single-wait NOPs placed just before, same engine."""
    for f in nc.m.functions:
        for bb in f.blocks:
            newi = []
            changed = False
            for ins in bb.instructions:
                si = ins.sync_info
                maxw = 1
                if si is not None and len(si.on_wait) > maxw:
                    waits = list(si.on_wait)
                    keep = waits[-maxw:]
                    for i, w in enumerate(waits[:-maxw]):
                        nop = mybir.InstNoOp(name=f"{ins.name}-ws{i}")
                        nop.engine = ins.engine
                        nop.sync_info = mybir.SyncInfo(on_wait=[w], on_update=[])
                        newi.append(nop)
                    ins.sync_info = mybir.SyncInfo(
                        on_wait=list(keep), on_update=list(si.on_update))
                    changed = True
                newi.append(ins)
            if changed:
                bb.instructions[:] = newi


_CACHE = {}


def _get_program():
    if "real" not in _CACHE:
        nc, d = build_program()
        # populate .instr bytes for extended gpsimd instructions
        # (sparse_gather, dma_gather, library reload) - raw Bass doesn't
        # run this codegen pass and walrus errors "ISA wrong length" without it
        from concourse.library_overlay import lower_extended_insts
        lower_extended_insts(nc)
        _split_excess_waits(nc)
        _CACHE["real"] = (nc, d)
    return _CACHE["real"]


def make_in_maps(x2d, W, d):
    """Host-side prep: f32+f16 token slices + per-core fused f16 tensor."""
    H, S = d["H"], d["S"]
    HP, SLOC, SG, DG = d["HP"], d["SLOC"], d["SG"], d["DG"]
    xt = x2d.T.astype(np.float16)          # [H, S]
    wt = W.T.astype(np.float16)            # [H, D]
    x16 = x2d.astype(np.float16)
    in_maps = []
    for c in range(N_CORES):
        sa, cb = divmod(c, 4)
        xw = np.zeros((HP, 2 * DG), np.float16)
        xw[:H, :DG] = xt[:, sa * SG:(sa + 1) * SG]
        xw[:H, DG:] = wt[:, cb * DG:(cb + 1) * DG]
        in_maps.append({
            "xs": np.ascontiguousarray(x2d[c * SLOC:(c + 1) * SLOC, :]),
            "xs16": np.ascontiguousarray(x16[c * SLOC:(c + 1) * SLOC, :]),
            "xw": xw,
        })
    return in_maps


def kernel(x, W):
    x = np.asarray(x)
    W = np.asarray(W)
    B, S, H = x.shape
    D = W.shape[0]
    assert (S, H, D) == (REAL["S"], REAL["H"], REAL["D"])
    nc, d = _get_program()
    in_maps = make_in_maps(x.reshape(S, H), W, d)
    res = run_bass_kernel_spmd(nc, in_maps, core_ids=list(range(N_CORES)))
    SG, DG = d["SG"], d["DG"]
    out = np.empty((S, D), np.float32)
    for c in range(N_CORES):
        sa, cb = divmod(c, 4)
        out[sa * SG:(sa + 1) * SG, cb * DG:(cb + 1) * DG] = res.results[c]["out"]
    return out.reshape(B, S, D)


# revision 23
# speedup vs baseline: 1.2899x; 1.1146x over previous
"""Trainium2 Bass kernel for nn_CustomMLPLayer_74526272520565 (topk_masking).

Reference semantics:
  core_idx = top-n_core neurons by how often they appear in each token's
             top-k_tok activations (count ties broken toward lower index)
  out = x[..., core_idx] @ W[:, core_idx].T

Distribution (8 NeuronCores): token-sharded counting (AllReduced), then
2D-sharded GEMM (2 token-halves x 4 col-blocks of W rows).

Per-core device algorithm:
  A. For its 256-token slice: exact k_tok-th largest activation per token.
     Bisection probes run on a host-provided f16 copy (DVE 4x mode / ScalarE
     Sign), bracketed by f16 stats; the finisher extracts the top-8 f32
     values of the f16-band and rank-selects the exact f32 threshold.
     sel = (x32 >= t*); counts[j] = sum_s sel[s, j] via PE matmuls.
  B. AllReduce counts; exact core-set threshold tau via integer bisection
     (DVE compare + gpsimd partition_all_reduce); tie selection among
     count==tau via prefix-scan rank (tensor_tensor_scan + triangular
     matmul), no second bisection.
  C. Compact the 4403 core indices (gpsimd sparse_gather) + 77 zero-row pads.
  D. 2D GEMM sharding: dma_gather rows of a host-fused f16 tensor
     [x^T half | W^T block] (one 4KB-row gather per K-tile feeds both
     operands); reduced GEMM (K=4480) accumulated in PSUM f32, two M-half
     sweeps of 8x [128,512] accumulators.
"""
import numpy as np

import concourse.bass as bass
import concourse.bass_isa as bass_isa
import concourse.mybir as mybir
from concourse.tile import TileContext
from concourse.tile_rust import add_dep_helper
from concourse import library_config
from concourse.bass_utils import run_bass_kernel_spmd

AF = mybir.ActivationFunctionType
OP = mybir.AluOpType
F32 = mybir.dt.float32
F16 = mybir.dt.float16
U8 = mybir.dt.uint8
I16 = mybir.dt.int16
U32 = mybir.dt.uint32

N_CORES = 8

REAL = dict(S=2048, H=11008, D=4096)
TOKEN_SPARSITY = 0.2
SPARSITY = 0.4

Z80 = 0.8416212335729143
ZLO = Z80 - 0.065
ZHI = Z80 + 0.080
N_BISECT = 9         # bisection iterations (host-simulated: max rank 5 <= 7)
SC_PROBES_T1 = 9     # tile-1 probes all on ScalarE (Sign); tile-0 all on DVE


def dims_for(S, H, D):
    assert H % 128 == 0 and H % 16 == 0 and D % N_CORES == 0
    d = {}
    d["S"], d["H"], d["D"] = S, H, D
    d["SLOC"] = S // N_CORES
    assert d["SLOC"] % 128 == 0
    d["NTT"] = d["SLOC"] // 128
    d["DLOC"] = D // N_CORES
    # GEMM 2D sharding: 2 token-halves x 4 col-blocks
    d["SG"] = S // 2
    d["DG"] = D // 4
    assert d["SG"] % 128 == 0 and d["DG"] % 128 == 0
    d["KTOK"] = int(H * TOKEN_SPARSITY)
    d["NCORE"] = int(H * SPARSITY)
    d["CH"] = H // 128
    d["NCP"] = ((d["NCORE"] + 127) // 128) * 128
    d["KT"] = d["NCP"] // 128
    d["HP"] = H + 128
    d["YF"] = H // 16
    d["NPAD"] = d["NCP"] - d["NCORE"]
    d["YP"] = (d["NPAD"] + 15) // 16
    assert 16 * d["YP"] <= 128
    d["CBITS"] = max(1, int(np.ceil(np.log2(S + 1))))
    return d


def build_program(S=REAL["S"], H=REAL["H"], D=REAL["D"]):
    d = dims_for(S, H, D)
    SLOC, NTT = d["SLOC"], d["NTT"]
    KTOK, NCORE, CH = d["KTOK"], d["NCORE"], d["CH"]
    NCP, KT, YF, NPAD, YP = d["NCP"], d["KT"], d["YF"], d["NPAD"], d["YP"]
    HP = d["HP"]
    CBITS = d["CBITS"]
    SG, DG = d["SG"], d["DG"]

    nc = bass.Bass("TRN2", num_devices=N_CORES)

    xs_d = nc.dram_tensor("xs", [SLOC, H], F32, kind="ExternalInput")
    xs16_d = nc.dram_tensor("xs16", [SLOC, H], F16, kind="ExternalInput")
    xw_d = nc.dram_tensor("xw", [HP, 2 * DG], F16, kind="ExternalInput")
    out_d = nc.dram_tensor("out", [SG, DG], F32, kind="ExternalOutput")
    NTT0 = S // N_CORES // 128
    cc_in = [nc.dram_tensor(f"cc_in{t}", [128, CH], F32) for t in range(NTT0)]
    cc_out = [nc.dram_tensor(f"cc_out{t}", [128, CH], F32,
                             addr_space="Shared") for t in range(NTT0)]

    with TileContext(nc) as tc:
        with tc.tile_pool(name="state", bufs=1) as st:
            ones16 = st.tile([128, 1], F16)
            nc.vector.memset(ones16[:], 1.0)
            io8 = st.tile([128, 8], F32)
            i_io8 = nc.gpsimd.iota(io8[:], pattern=[[1, 8]], base=0,
                                   channel_multiplier=0,
                                   allow_small_or_imprecise_dtypes=True)
            compR = st.tile([128, NCP // 16], I16, tag="compR")
            # iotas for phase B/C (issued early; all standard-lib gpsimd ops
            # must precede the first library load)
            jy1 = st.tile([16, YF], F32, tag="jy1")
            i_jy = nc.gpsimd.iota(jy1[:], pattern=[[1, YF]], base=1,
                                  channel_multiplier=YF,
                                  allow_small_or_imprecise_dtypes=True)
            pv = st.tile([16, YP], F32, tag="pv")
            i_pv = nc.gpsimd.iota(pv[:], pattern=[[1, YP]], base=H,
                                  channel_multiplier=YP,
                                  allow_small_or_imprecise_dtypes=True)
            # strict upper-triangular [16,16] ones: UT[p, c] = (p < c)
            itr = st.tile([16, 16], F32, tag="itr")
            i_itr = nc.gpsimd.iota(itr[:], pattern=[[0, 16]], base=0,
                                   channel_multiplier=1,
                                   allow_small_or_imprecise_dtypes=True)
            itc = st.tile([16, 16], F32, tag="itc")
            i_itc = nc.gpsimd.iota(itc[:], pattern=[[1, 16]], base=0,
                                   channel_multiplier=0,
                                   allow_small_or_imprecise_dtypes=True)
            ut16 = st.tile([16, 16], F32, tag="ut16")
            nc.vector.tensor_tensor(out=ut16[:], in0=itr[:], in1=itc[:],
                                    op=OP.is_lt)
            iota_insts = [i_io8, i_jy, i_pv, i_itr, i_itc]

            with tc.tile_pool(name="cnt", bufs=1) as cp, \
                 tc.tile_pool(name="psc", bufs=1, space="PSUM") as psc, \
                 tc.tile_pool(name="pss", bufs=1, space="PSUM") as pss:

                # ---------- phase A: per-token thresholds, sel, counts --------
                x16_t = [cp.tile([128, H], F16, tag=f"x16{t}", name=f"x16_t{t}")
                         for t in range(NTT)]
                x32_t = [cp.tile([128, H], F32, tag=f"x32{t}", name=f"x32_t{t}")
                         for t in range(NTT)]
                # probe masks are never read: stride-0 sink APs (no SBUF cost)
                vsink = st.tile([128, 1], F16, tag="vsink")
                ssink = st.tile([128, 1], F16, tag="ssink")
                vsink_b = vsink[:, 0:1].broadcast_to((128, H))
                ssink_b = ssink[:, 0:1].broadcast_to((128, H))
                psum_cnt = psc.tile([128, CH], F32)
                for t in range(NTT):
                    nc.sync.dma_start(x16_t[t][:],
                                      xs16_d[t * 128:(t + 1) * 128, :])
                for t in range(NTT):
                    nc.sync.dma_start(x32_t[t][:],
                                      xs_d[t * 128:(t + 1) * 128, :])

                A_t, B_t, CB_t, TS_t = [], [], [], []
                for t in range(NTT):
                    A_t.append(st.tile([128, 1], F32, tag=f"A{t}", name=f"A{t}"))
                    B_t.append(st.tile([128, 1], F32, tag=f"B{t}", name=f"B{t}"))
                    CB_t.append(st.tile([128, 1], F32, tag=f"CB{t}",
                                        name=f"CB{t}"))
                    TS_t.append(st.tile([128, 1], F32, tag=f"TS{t}",
                                        name=f"TS{t}"))

                # stats: tile-0 mean on DVE (its probes wait on the bracket
                # anyway); everything else on ScalarE
                for t in range(NTT):
                    s1 = st.tile([128, 1], F32, tag=f"s1{t}")
                    s2 = st.tile([128, 1], F32, tag=f"s2{t}")
                    if t == 0:
                        nc.vector.tensor_scalar(vsink_b, x16_t[t][:], 0.0,
                                                None, op0=OP.add, op1=OP.add,
                                                accum_out=s1[:])
                    else:
                        nc.scalar.activation(ssink_b, x16_t[t][:], AF.Copy,
                                             accum_out=s1[:])
                    nc.scalar.activation(ssink_b, x16_t[t][:], AF.Square,
                                         0.0, 1.0, 0.0, accum_out=s2[:])
                    mu = st.tile([128, 1], F32, tag=f"mu{t}")
                    var = st.tile([128, 1], F32, tag=f"var{t}")
                    sig = st.tile([128, 1], F32, tag=f"sig{t}")
                    musq = st.tile([128, 1], F32, tag=f"musq{t}")
                    nc.vector.tensor_scalar_mul(mu[:], s1[:], 1.0 / H)
                    nc.vector.tensor_scalar_mul(var[:], s2[:], 1.0 / H)
                    nc.vector.tensor_tensor(out=musq[:], in0=mu[:], in1=mu[:],
                                            op=OP.mult)
                    nc.vector.tensor_tensor(out=var[:], in0=var[:],
                                            in1=musq[:], op=OP.subtract)
                    nc.scalar.sqrt(sig[:], var[:])
                    nc.vector.scalar_tensor_tensor(A_t[t][:], sig[:], ZLO,
                                                   mu[:], op0=OP.mult,
                                                   op1=OP.add)
                    nc.vector.scalar_tensor_tensor(B_t[t][:], sig[:], ZHI,
                                                   mu[:], op0=OP.mult,
                                                   op1=OP.add)
                    nc.vector.memset(CB_t[t][:], 0.0)

                def probe_dve16(t, thr_ap, cout_ap):
                    nc.vector.tensor_scalar(vsink_b, x16_t[t][:], thr_ap,
                                            None, op0=OP.is_gt, op1=OP.add,
                                            accum_out=cout_ap)

                def probe_act16(t, thr_ap, cout_ap):
                    nthr = st.tile([128, 1], F32, tag="nthr")
                    nc.vector.tensor_scalar_mul(nthr[:], thr_ap, -1.0)
                    acc = st.tile([128, 1], F32, tag="acc")
                    nc.scalar.activation(ssink_b, x16_t[t][:], AF.Sign,
                                         bias=nthr[:], scale=1.0,
                                         accum_out=acc[:])
                    nc.vector.tensor_scalar(cout_ap, acc[:], float(H), 0.5,
                                            op0=OP.add, op1=OP.mult)

                c_pr = [st.tile([128, 1], F32, tag=f"cpr{t}", name=f"cpr{t}")
                        for t in range(NTT)]
                tmid = [st.tile([128, 1], F32, tag=f"tmid{t}", name=f"tmid{t}")
                        for t in range(NTT)]
                mge = st.tile([128, 1], U8, tag="mge")
                mlt = st.tile([128, 1], U8, tag="mlt")

                def bis_update(t, c_ap, mid_ap):
                    nc.vector.tensor_scalar(mge[:], c_ap, float(KTOK), None,
                                            op0=OP.is_ge)
                    nc.vector.copy_predicated(A_t[t][:], mge[:], mid_ap)
                    nc.vector.tensor_scalar(mlt[:], c_ap, float(KTOK), None,
                                            op0=OP.is_lt)
                    nc.vector.copy_predicated(B_t[t][:], mlt[:], mid_ap)
                    nc.vector.copy_predicated(CB_t[t][:], mlt[:], c_ap)

                for it in range(N_BISECT):
                    for t in range(NTT):
                        nc.vector.tensor_tensor(out=tmid[t][:], in0=A_t[t][:],
                                                in1=B_t[t][:], op=OP.add)
                        nc.vector.tensor_scalar_mul(tmid[t][:], tmid[t][:],
                                                    0.5)
                        if t % 2 == 1 and it < SC_PROBES_T1:
                            probe_act16(t, tmid[t][:], c_pr[t][:])
                        else:
                            probe_dve16(t, tmid[t][:], c_pr[t][:])
                        bis_update(t, c_pr[t][:], tmid[t][:])

                # finisher: t* = (KTOK - CB)-th largest f32 among {x16 <= B};
                # sel threshold = midpoint between ranks rm1 and rm1+1 (never
                # equals a data value, so the ScalarE Sign sel is exact)
                yband = cp.tile([128, H], F32, tag="yband")
                cnt_t = [cp.tile([128, CH], F32, tag=f"cnt{t}",
                                 name=f"cnt_t{t}") for t in range(NTT)]
                i_cc = []
                for t in range(NTT):
                    nc.vector.scalar_tensor_tensor(yband[:], x16_t[t][:],
                                                   B_t[t][:], x32_t[t][:],
                                                   op0=OP.is_le, op1=OP.mult)
                    m8 = st.tile([128, 8], F32, tag=f"m8{t}")
                    nc.vector.max(out=m8[:], in_=yband[:])
                    rm1 = st.tile([128, 1], F32, tag=f"rm1{t}")
                    nc.vector.tensor_scalar(rm1[:], CB_t[t][:],
                                            float(-(KTOK - 1)), -1.0,
                                            op0=OP.add, op1=OP.mult)
                    rm1p = st.tile([128, 1], F32, tag=f"rm1p{t}")
                    nc.vector.tensor_scalar(rm1p[:], rm1[:], 1.0, None,
                                            op0=OP.add)
                    rm1q = st.tile([128, 1], F32, tag=f"rm1q{t}")
                    nc.vector.tensor_scalar(rm1q[:], rm1[:], 2.0, None,
                                            op0=OP.add)
                    # windowed rank match (robust to +-0.5 CB offset from the
                    # ScalarE sign-count path): pick i = ceil(rm1), and the
                    # next rank for the midpoint
                    sel8 = st.tile([128, 8], F32, tag=f"sel8{t}")
                    nc.vector.scalar_tensor_tensor(sel8[:], io8[:], rm1[:],
                                                   m8[:], op0=OP.is_ge,
                                                   op1=OP.mult)
                    sel8b = st.tile([128, 8], F32, tag=f"sel8b{t}")
                    nc.vector.scalar_tensor_tensor(sel8b[:], io8[:], rm1p[:],
                                                   sel8[:], op0=OP.is_lt,
                                                   op1=OP.mult,
                                                   accum_out=TS_t[t][:])
                    sel8c = st.tile([128, 8], F32, tag=f"sel8c{t}")
                    nc.vector.scalar_tensor_tensor(sel8c[:], io8[:], rm1p[:],
                                                   m8[:], op0=OP.is_ge,
                                                   op1=OP.mult)
                    ts2 = st.tile([128, 1], F32, tag=f"ts2{t}")
                    sel8d = st.tile([128, 8], F32, tag=f"sel8d{t}")
                    nc.vector.scalar_tensor_tensor(sel8d[:], io8[:], rm1q[:],
                                                   sel8c[:], op0=OP.is_lt,
                                                   op1=OP.mult,
                                                   accum_out=ts2[:])
                    # bias = -(TS + TS2)/2 for the Sign sel
                    nmid = st.tile([128, 1], F32, tag=f"nmid{t}")
                    nc.vector.tensor_tensor(out=nmid[:], in0=TS_t[t][:],
                                            in1=ts2[:], op=OP.add)
                    nc.vector.tensor_scalar_mul(nmid[:], nmid[:], -0.5)
                    # sel (+-1 valued) on ScalarE, reusing x16_t[t]'s buffer
                    sel = cp.tile([128, H], F16, tag=f"x16{t}",
                                  name=f"sel{t}")
                    nc.scalar.activation(sel[:], x32_t[t][:], AF.Sign,
                                         bias=nmid[:], scale=1.0)
                    for f in range(CH):
                        nc.tensor.matmul(psum_cnt[:, f:f + 1], sel[:, f::CH],
                                         ones16[:], start=True, stop=True)
                    # counts = (sum(+-1) + 128) / 2
                    nc.vector.tensor_scalar(cnt_t[t][:], psum_cnt[:], 128.0,
                                            0.5, op0=OP.add, op1=OP.mult)
                    nc.sync.dma_start(cc_in[t][:], cnt_t[t][:])
                    cc = nc.gpsimd.collective_compute(
                        "AllReduce", OP.add,
                        replica_groups=[[i for i in range(N_CORES)]],
                        ins=[cc_in[t][:].opt()], outs=[cc_out[t][:].opt()],
                    )
                    i_cc.append(cc)

                # mlp library (partition_all_reduce) for phase B; load
                # overlaps the AllReduce wait
                i_lib_m1 = nc.gpsimd.load_library(library_config.mlp)
                for dep in iota_insts:
                    add_dep_helper(i_lib_m1.ins, dep.ins, sync=False,
                                   reason="lib order")
                add_dep_helper(i_lib_m1.ins, i_cc[-1].ins, sync=False,
                               reason="lib order")

                # ---------- phase B: tau bisection + prefix-scan ties ---------
                call = cp.tile([128, CH], F32, tag="call")
                callb = cp.tile([128, CH], F32, tag="callb")
                nc.sync.dma_start(call[:], cc_out[0][:])
                nc.sync.dma_start(callb[:], cc_out[1][:])
                nc.vector.tensor_tensor(out=call[:], in0=call[:],
                                        in1=callb[:], op=OP.add)
                ycnt = cp.tile([16, YF], F32, tag="ycnt")
                ycntb = cp.tile([16, YF], F32, tag="ycntb")
                nc.sync.dma_start(ycnt[:],
                                  cc_out[0][:].rearrange("(a b) c -> a (b c)",
                                                         a=16))
                nc.sync.dma_start(ycntb[:],
                                  cc_out[1][:].rearrange("(a b) c -> a (b c)",
                                                         a=16))
                nc.vector.tensor_tensor(out=ycnt[:], in0=ycnt[:],
                                        in1=ycntb[:], op=OP.add)

                scr86 = cp.tile([128, CH], U8, tag="scr86")
                gpart = st.tile([128, 1], F32, tag="gpart")
                Gb = st.tile([128, 1], F32, tag="Gb")

                lo = st.tile([128, 1], F32, tag="lo")
                hi = st.tile([128, 1], F32, tag="hi")
                Ghi = st.tile([128, 1], F32, tag="Ghi")
                mid = st.tile([128, 1], F32, tag="mid")
                nc.vector.memset(lo[:], -0.5)
                nc.vector.memset(hi[:], 2.0 ** CBITS - 0.5)
                nc.vector.memset(Ghi[:], 0.0)
                last_par = i_lib_m1
                for it in range(CBITS):
                    nc.vector.tensor_tensor(out=mid[:], in0=lo[:], in1=hi[:],
                                            op=OP.add)
                    nc.vector.tensor_scalar_mul(mid[:], mid[:], 0.5)
                    nc.vector.tensor_scalar(scr86[:], call[:], mid[:], None,
                                            op0=OP.is_gt, op1=OP.add,
                                            accum_out=gpart[:])
                    par = nc.gpsimd.partition_all_reduce(
                        Gb[:], gpart[:], channels=128,
                        reduce_op=bass_isa.ReduceOp.add)
                    add_dep_helper(par.ins, last_par.ins, sync=False,
                                   reason="lib order")
                    last_par = par
                    nc.vector.tensor_scalar(mge[:], Gb[:], float(NCORE), None,
                                            op0=OP.is_ge)
                    nc.vector.copy_predicated(lo[:], mge[:], mid[:])
                    nc.vector.tensor_scalar(mlt[:], Gb[:], float(NCORE), None,
                                            op0=OP.is_lt)
                    nc.vector.copy_predicated(hi[:], mlt[:], mid[:])
                    nc.vector.copy_predicated(Ghi[:], mlt[:], Gb[:])
                tau = st.tile([128, 1], F32, tag="tau")
                nc.vector.tensor_scalar(tau[:], lo[:], 0.5, None, op0=OP.add)
                rr = st.tile([128, 1], F32, tag="rr")
                nc.vector.tensor_scalar(rr[:], Ghi[:], float(-NCORE), -1.0,
                                        op0=OP.add, op1=OP.mult)

                # ---------- phase C: y build (prefix ties) + sparse_gather ----
                y = cp.tile([16, YF + YP], F32, tag="y")
                ties = cp.tile([16, YF], F32, tag="ties")
                nc.vector.tensor_scalar(ties[:], ycnt[:], tau[:16, :], None,
                                        op0=OP.is_equal)
                zz = cp.tile([16, YF], F32, tag="zz")
                nc.vector.memset(zz[:], 0.0)
                pfx = cp.tile([16, YF], F32, tag="pfx")
                nc.vector.tensor_tensor_scan(pfx[:], ties[:], zz[:], 0.0,
                                             op0=OP.add, op1=OP.add)
                # cross-partition exclusive offsets of tie totals
                pso16 = pss.tile([16, 1], F32, tag="pso16")
                nc.tensor.matmul(pso16[:], ut16[:], pfx[:, YF - 1:YF],
                                 start=True, stop=True)
                off = st.tile([16, 1], F32, tag="off")
                nc.vector.tensor_copy(off[:], pso16[:])
                rnk = cp.tile([16, YF], F32, tag="rnk")
                nc.vector.tensor_scalar(rnk[:], pfx[:], off[:], None,
                                        op0=OP.add)
                msel = cp.tile([16, YF], F32, tag="msel")
                nc.vector.scalar_tensor_tensor(msel[:], rnk[:], rr[:16, :],
                                               ties[:], op0=OP.is_le,
                                               op1=OP.mult)
                c1y = cp.tile([16, YF], F32, tag="c1y")
                nc.vector.tensor_scalar(c1y[:], ycnt[:], tau[:16, :], None,
                                        op0=OP.is_gt)
                nc.vector.tensor_tensor(out=c1y[:], in0=c1y[:], in1=msel[:],
                                        op=OP.add)
                nc.vector.tensor_tensor(out=y[:, :YF], in0=c1y[:], in1=jy1[:],
                                        op=OP.mult)
                nc.vector.tensor_scalar(y[:, :YF], y[:, :YF], -1.0, None,
                                        op0=OP.add)
                pm = cp.tile([16, YP], F32, tag="pm")
                nc.vector.tensor_scalar(pm[:], pv[:], float(H + NPAD - 1),
                                        None, op0=OP.is_le)
                pv1 = cp.tile([16, YP], F32, tag="pv1")
                nc.vector.tensor_scalar(pv1[:], pv[:], 1.0, None, op0=OP.add)
                nc.vector.tensor_tensor(out=y[:, YF:], in0=pm[:], in1=pv1[:],
                                        op=OP.mult)
                nc.vector.tensor_scalar(y[:, YF:], y[:, YF:], -1.0, None,
                                        op0=OP.add)

                comp = cp.tile([16, NCP // 16], F32, tag="comp")
                nfound = st.tile([1, 1], U32, tag="nfound")
                i_lib8 = nc.gpsimd.load_library(library_config.sparse_gather)
                add_dep_helper(i_lib8.ins, last_par.ins, sync=False,
                               reason="lib order")
                i_sg = nc.gpsimd.sparse_gather(comp[:], y[:],
                                               num_found=nfound[:])
                add_dep_helper(i_sg.ins, i_lib8.ins, sync=False,
                               reason="lib order")

                comp16 = cp.tile([16, NCP // 16], I16, tag="comp16")
                nc.vector.tensor_copy(comp16[:], comp[:])
                for r in range(8):
                    nc.sync.dma_start(compR[16 * r:16 * r + 16, :], comp16[:])

            # ---------- phase D: fused gathers + reduced GEMM -----------------
            i_lib3 = nc.gpsimd.load_library(library_config.mlp)
            add_dep_helper(i_lib3.ins, i_sg.ins, sync=False, reason="lib order")

            with tc.tile_pool(name="gemm", bufs=1) as gp, \
                 tc.tile_pool(name="outp", bufs=3) as op_, \
                 tc.tile_pool(name="pso", bufs=1, space="PSUM") as pso:
                xw_t = [gp.tile([128, 1, 2 * DG], F16, tag=f"xw{kt}",
                                name=f"xw{kt}") for kt in range(KT)]
                prev = i_lib3
                n128_reg = nc.gpsimd.to_reg(128)
                for kt in range(KT):
                    ix = compR[:, 8 * kt:8 * kt + 8]
                    g = nc.gpsimd.dma_gather(xw_t[kt][:], xw_d[:], ix,
                                             num_idxs=128,
                                             num_idxs_reg=n128_reg,
                                             elem_size=2 * DG)
                    add_dep_helper(g.ins, prev.ins, sync=False,
                                   reason="lib order")
                    prev = g

                MT = SG // 128
                MH = MT // 2
                for half in range(2):
                    ptiles = [pso.tile([128, 512], F32, tag=f"po{i}",
                                       name=f"po{half}_{i}")
                              for i in range(2 * MH)]
                    for kt in range(KT):
                        for i in range(MH):
                            m = half * MH + i
                            for n in range(2):
                                nc.tensor.matmul(
                                    ptiles[2 * i + n][:],
                                    xw_t[kt][:, 0, 128 * m:128 * (m + 1)],
                                    xw_t[kt][:, 0,
                                             DG + 512 * n:DG + 512 * (n + 1)],
                                    start=(kt == 0), stop=(kt == KT - 1))
                    for i in range(MH):
                        m = half * MH + i
                        outs = op_.tile([128, DG], F32, tag="outs")
                        if i % 2 == 0:
                            nc.vector.tensor_copy(outs[:, :512],
                                                  ptiles[2 * i][:])
                            nc.vector.tensor_copy(outs[:, 512:],
                                                  ptiles[2 * i + 1][:])
                        else:
                            nc.scalar.copy(outs[:, :512], ptiles[2 * i][:])
                            nc.scalar.copy(outs[:, 512:], ptiles[2 * i + 1][:])
                        nc.sync.dma_start(out_d[128 * m:128 * (m + 1), :],
                                          outs[:])

    return nc, d


def _split_excess_waits(nc):
    """This walrus build rejects >1 sync wait on several instruction structs;
    hoist extra waits into single-wait NOPs placed just before, same engine."""
    for f in nc.m.functions:
        for bb in f.blocks:
            newi = []
            changed = False
            for ins in bb.instructions:
                si = ins.sync_info
                maxw = 1
                if si is not None and len(si.on_wait) > maxw:
                    waits = list(si.on_wait)
                    keep = waits[-maxw:]
                    for i, w in enumerate(waits[:-maxw]):
                        nop = mybir.InstNoOp(name=f"{ins.name}-ws{i}")
                        nop.engine = ins.engine
                        nop.sync_info = mybir.SyncInfo(on_wait=[w], on_update=[])
                        newi.append(nop)
                    ins.sync_info = mybir.SyncInfo(
                        on_wait=list(keep), on_update=list(si.on_update))
                    changed = True
                newi.append(ins)
            if changed:
                bb.instructions[:] = newi


_CACHE = {}


def _get_program():
    if "real" not in _CACHE:
        nc, d = build_program()
        # populate .instr bytes for extended gpsimd instructions
        # (sparse_gather, dma_gather, library reload) - raw Bass doesn't
        # run this codegen pass and walrus errors "ISA wrong length" without it
        from concourse.library_overlay import lower_extended_insts
        lower_extended_insts(nc)
        _split_excess_waits(nc)
        _CACHE["real"] = (nc, d)
    return _CACHE["real"]


def make_in_maps(x2d, W, d):
    """Host-side prep: f32+f16 token slices + per-core fused f16 tensor."""
    H, S = d["H"], d["S"]
    HP, SLOC, SG, DG = d["HP"], d["SLOC"], d["SG"], d["DG"]
    xt = x2d.T.astype(np.float16)          # [H, S]
    wt = W.T.astype(np.float16)            # [H, D]
    x16 = x2d.astype(np.float16)
    in_maps = []
    for c in range(N_CORES):
        sa, cb = divmod(c, 4)
        xw = np.zeros((HP, 2 * DG), np.float16)
        xw[:H, :DG] = xt[:, sa * SG:(sa + 1) * SG]
        xw[:H, DG:] = wt[:, cb * DG:(cb + 1) * DG]
        in_maps.append({
            "xs": np.ascontiguousarray(x2d[c * SLOC:(c + 1) * SLOC, :]),
            "xs16": np.ascontiguousarray(x16[c * SLOC:(c + 1) * SLOC, :]),
            "xw": xw,
        })
    return in_maps


def kernel(x, W):
    x = np.asarray(x)
    W = np.asarray(W)
    B, S, H = x.shape
    D = W.shape[0]
    assert (S, H, D) == (REAL["S"], REAL["H"], REAL["D"])
    nc, d = _get_program()
    in_maps = make_in_maps(x.reshape(S, H), W, d)
    res = run_bass_kernel_spmd(nc, in_maps, core_ids=list(range(N_CORES)))
    SG, DG = d["SG"], d["DG"]
    out = np.empty((S, D), np.float32)
    for c in range(N_CORES):
        sa, cb = divmod(c, 4)
        out[sa * SG:(sa + 1) * SG, cb * DG:(cb + 1) * DG] = res.results[c]["out"]
    return out.reshape(B, S, D)


# revision 29
# speedup vs baseline: 1.3358x; 1.0356x over previous
"""Trainium2 Bass kernel for nn_CustomMLPLayer_74526272520565 (topk_masking).

Reference semantics:
  core_idx = top-n_core neurons by how often they appear in each token's
             top-k_tok activations (count ties broken toward lower index)
  out = x[..., core_idx] @ W[:, core_idx].T

Distribution (8 NeuronCores): token-sharded counting (AllReduced), then
2D-sharded GEMM (2 token-halves x 4 col-blocks of W rows).

Per-core device algorithm:
  A. For its 256-token slice: exact k_tok-th largest activation per token.
     Bisection probes run on a host-provided f16 copy (DVE 4x mode / ScalarE
     Sign), bracketed by f16 stats; the finisher extracts the top-8 f32
     values of the f16-band and rank-selects the exact f32 threshold.
     sel = (x32 >= t*); counts[j] = sum_s sel[s, j] via PE matmuls.
  B. AllReduce counts; exact core-set threshold tau via integer bisection
     (DVE compare + gpsimd partition_all_reduce); tie selection among
     count==tau via prefix-scan rank (tensor_tensor_scan + triangular
     matmul), no second bisection.
  C. Compact the 4403 core indices (gpsimd sparse_gather) + 77 zero-row pads.
  D. 2D GEMM sharding: dma_gather rows of a host-fused f16 tensor
     [x^T half | W^T block] (one 4KB-row gather per K-tile feeds both
     operands); reduced GEMM (K=4480) accumulated in PSUM f32, two M-half
     sweeps of 8x [128,512] accumulators.
"""
import numpy as np

import concourse.bass as bass
import concourse.bass_isa as bass_isa
import concourse.mybir as mybir
from concourse.tile import TileContext
from concourse.tile_rust import add_dep_helper
from concourse import library_config
from concourse.bass_utils import run_bass_kernel_spmd

AF = mybir.ActivationFunctionType
OP = mybir.AluOpType
F32 = mybir.dt.float32
F16 = mybir.dt.float16
U8 = mybir.dt.uint8
I16 = mybir.dt.int16
U32 = mybir.dt.uint32

N_CORES = 8

REAL = dict(S=2048, H=11008, D=4096)
TOKEN_SPARSITY = 0.2
SPARSITY = 0.4

Z80 = 0.8416212335729143
ZLO = Z80 - 0.065
ZHI = Z80 + 0.080
N_BISECT = 9         # bisection iterations (host-simulated: max rank 5 <= 7)
SC_PROBES_T1 = 9     # tile-1 probes all on ScalarE (Sign); tile-0 all on DVE


def dims_for(S, H, D):
    assert H % 128 == 0 and H % 16 == 0 and D % N_CORES == 0
    d = {}
    d["S"], d["H"], d["D"] = S, H, D
    d["SLOC"] = S // N_CORES
    assert d["SLOC"] % 128 == 0
    d["NTT"] = d["SLOC"] // 128
    d["DLOC"] = D // N_CORES
    # GEMM 2D sharding: 2 token-halves x 4 col-blocks
    d["SG"] = S // 2
    d["DG"] = D // 4
    assert d["SG"] % 128 == 0 and d["DG"] % 128 == 0
    d["KTOK"] = int(H * TOKEN_SPARSITY)
    d["NCORE"] = int(H * SPARSITY)
    d["CH"] = H // 128
    d["NCP"] = ((d["NCORE"] + 127) // 128) * 128
    d["KT"] = d["NCP"] // 128
    d["HP"] = H + 128
    d["YF"] = H // 16
    d["NPAD"] = d["NCP"] - d["NCORE"]
    d["YP"] = (d["NPAD"] + 15) // 16
    assert 16 * d["YP"] <= 128
    d["CBITS"] = max(1, int(np.ceil(np.log2(S + 1))))
    return d


def build_program(S=REAL["S"], H=REAL["H"], D=REAL["D"]):
    d = dims_for(S, H, D)
    SLOC, NTT = d["SLOC"], d["NTT"]
    KTOK, NCORE, CH = d["KTOK"], d["NCORE"], d["CH"]
    NCP, KT, YF, NPAD, YP = d["NCP"], d["KT"], d["YF"], d["NPAD"], d["YP"]
    HP = d["HP"]
    CBITS = d["CBITS"]
    SG, DG = d["SG"], d["DG"]

    nc = bass.Bass("TRN2", num_devices=N_CORES)

    xs_d = nc.dram_tensor("xs", [SLOC, H], F32, kind="ExternalInput")
    xs16_d = nc.dram_tensor("xs16", [SLOC, H], F16, kind="ExternalInput")
    xw_d = nc.dram_tensor("xw", [HP, 2 * DG], F16, kind="ExternalInput")
    out_d = nc.dram_tensor("out", [SG, DG], F32, kind="ExternalOutput")
    NTT0 = S // N_CORES // 128
    cc_in = [nc.dram_tensor(f"cc_in{t}", [128, CH], F32) for t in range(NTT0)]
    cc_out = [nc.dram_tensor(f"cc_out{t}", [128, CH], F32,
                             addr_space="Shared") for t in range(NTT0)]

    with TileContext(nc) as tc:
        with tc.tile_pool(name="state", bufs=1) as st:
            ones16 = st.tile([128, 1], F16)
            nc.vector.memset(ones16[:], 1.0)
            io8 = st.tile([128, 8], F32)
            i_io8 = nc.gpsimd.iota(io8[:], pattern=[[1, 8]], base=0,
                                   channel_multiplier=0,
                                   allow_small_or_imprecise_dtypes=True)
            compR = st.tile([128, NCP // 16], I16, tag="compR")
            # iotas for phase B/C (issued early; all standard-lib gpsimd ops
            # must precede the first library load)
            jy1 = st.tile([16, YF], F32, tag="jy1")
            i_jy = nc.gpsimd.iota(jy1[:], pattern=[[1, YF]], base=1,
                                  channel_multiplier=YF,
                                  allow_small_or_imprecise_dtypes=True)
            pv = st.tile([16, YP], F32, tag="pv")
            i_pv = nc.gpsimd.iota(pv[:], pattern=[[1, YP]], base=H,
                                  channel_multiplier=YP,
                                  allow_small_or_imprecise_dtypes=True)
            # strict upper-triangular [16,16] ones: UT[p, c] = (p < c)
            itr = st.tile([16, 16], F32, tag="itr")
            i_itr = nc.gpsimd.iota(itr[:], pattern=[[0, 16]], base=0,
                                   channel_multiplier=1,
                                   allow_small_or_imprecise_dtypes=True)
            itc = st.tile([16, 16], F32, tag="itc")
            i_itc = nc.gpsimd.iota(itc[:], pattern=[[1, 16]], base=0,
                                   channel_multiplier=0,
                                   allow_small_or_imprecise_dtypes=True)
            ut16 = st.tile([16, 16], F32, tag="ut16")
            nc.vector.tensor_tensor(out=ut16[:], in0=itr[:], in1=itc[:],
                                    op=OP.is_lt)
            iota_insts = [i_io8, i_jy, i_pv, i_itr, i_itc]

            with tc.tile_pool(name="cnt", bufs=1) as cp, \
                 tc.tile_pool(name="psc", bufs=1, space="PSUM") as psc, \
                 tc.tile_pool(name="pss", bufs=1, space="PSUM") as pss:

                # ---------- phase A: per-token thresholds, sel, counts --------
                x16_t = [cp.tile([128, H], F16, tag=f"x16{t}", name=f"x16_t{t}")
                         for t in range(NTT)]
                x32_t = [cp.tile([128, H], F32, tag=f"x32{t}", name=f"x32_t{t}")
                         for t in range(NTT)]
                # probe masks are never read: stride-0 sink APs (no SBUF cost)
                vsink = st.tile([128, 1], F16, tag="vsink")
                ssink = st.tile([128, 1], F16, tag="ssink")
                vsink_b = vsink[:, 0:1].broadcast_to((128, H))
                ssink_b = ssink[:, 0:1].broadcast_to((128, H))
                psum_cnt = psc.tile([128, CH], F32)
                for t in range(NTT):
                    nc.sync.dma_start(x16_t[t][:],
                                      xs16_d[t * 128:(t + 1) * 128, :])
                for t in range(NTT):
                    nc.sync.dma_start(x32_t[t][:],
                                      xs_d[t * 128:(t + 1) * 128, :])

                A_t, B_t, CB_t, TS_t = [], [], [], []
                for t in range(NTT):
                    A_t.append(st.tile([128, 1], F32, tag=f"A{t}", name=f"A{t}"))
                    B_t.append(st.tile([128, 1], F32, tag=f"B{t}", name=f"B{t}"))
                    CB_t.append(st.tile([128, 1], F32, tag=f"CB{t}",
                                        name=f"CB{t}"))
                    TS_t.append(st.tile([128, 1], F32, tag=f"TS{t}",
                                        name=f"TS{t}"))

                # stats: tile-0 mean on DVE (its probes wait on the bracket
                # anyway); everything else on ScalarE
                for t in range(NTT):
                    s1 = st.tile([128, 1], F32, tag=f"s1{t}")
                    s2 = st.tile([128, 1], F32, tag=f"s2{t}")
                    if t == 0:
                        nc.vector.tensor_scalar(vsink_b, x16_t[t][:], 0.0,
                                                None, op0=OP.add, op1=OP.add,
                                                accum_out=s1[:])
                    else:
                        nc.scalar.activation(ssink_b, x16_t[t][:], AF.Copy,
                                             accum_out=s1[:])
                    nc.scalar.activation(ssink_b, x16_t[t][:], AF.Square,
                                         0.0, 1.0, 0.0, accum_out=s2[:])
                    mu = st.tile([128, 1], F32, tag=f"mu{t}")
                    var = st.tile([128, 1], F32, tag=f"var{t}")
                    sig = st.tile([128, 1], F32, tag=f"sig{t}")
                    musq = st.tile([128, 1], F32, tag=f"musq{t}")
                    nc.vector.tensor_scalar_mul(mu[:], s1[:], 1.0 / H)
                    nc.vector.tensor_scalar_mul(var[:], s2[:], 1.0 / H)
                    nc.vector.tensor_tensor(out=musq[:], in0=mu[:], in1=mu[:],
                                            op=OP.mult)
                    nc.vector.tensor_tensor(out=var[:], in0=var[:],
                                            in1=musq[:], op=OP.subtract)
                    nc.scalar.sqrt(sig[:], var[:])
                    nc.vector.scalar_tensor_tensor(A_t[t][:], sig[:], ZLO,
                                                   mu[:], op0=OP.mult,
                                                   op1=OP.add)
                    nc.vector.scalar_tensor_tensor(B_t[t][:], sig[:], ZHI,
                                                   mu[:], op0=OP.mult,
                                                   op1=OP.add)
                    nc.vector.memset(CB_t[t][:], 0.0)

                def probe_dve16(t, thr_ap, cout_ap):
                    return nc.vector.tensor_scalar(vsink_b, x16_t[t][:],
                                                   thr_ap, None, op0=OP.is_gt,
                                                   op1=OP.add,
                                                   accum_out=cout_ap)

                def probe_act16_pre(t, thr_ap):
                    nthr = st.tile([128, 1], F32, tag="nthr")
                    i_n = nc.vector.tensor_scalar_mul(nthr[:], thr_ap, -1.0)
                    acc = st.tile([128, 1], F32, tag="acc")
                    nc.scalar.activation(ssink_b, x16_t[t][:], AF.Sign,
                                         bias=nthr[:], scale=1.0,
                                         accum_out=acc[:])
                    return acc, i_n

                def probe_act16_post(acc, cout_ap):
                    nc.vector.tensor_scalar(cout_ap, acc[:], float(H), 0.5,
                                            op0=OP.add, op1=OP.mult)

                c_pr = [st.tile([128, 1], F32, tag=f"cpr{t}", name=f"cpr{t}")
                        for t in range(NTT)]
                tmid = [st.tile([128, 1], F32, tag=f"tmid{t}", name=f"tmid{t}")
                        for t in range(NTT)]
                mge = st.tile([128, 1], U8, tag="mge")
                mlt = st.tile([128, 1], U8, tag="mlt")

                def bis_update(t, c_ap, mid_ap):
                    nc.vector.tensor_scalar(mge[:], c_ap, float(KTOK), None,
                                            op0=OP.is_ge)
                    nc.vector.copy_predicated(A_t[t][:], mge[:], mid_ap)
                    nc.vector.tensor_scalar(mlt[:], c_ap, float(KTOK), None,
                                            op0=OP.is_lt)
                    nc.vector.copy_predicated(B_t[t][:], mlt[:], mid_ap)
                    nc.vector.copy_predicated(CB_t[t][:], mlt[:], c_ap)

                # per iteration: tile-1's threshold-negate + ScalarE Sign are
                # emitted first, and tile-0's (next) DVE probe depends on the
                # negate so the Tile list-scheduler cannot starve tile-1's
                # ScalarE chain behind back-to-back 11.7us DVE probes.
                i_nthr_prev = None
                for it in range(N_BISECT):
                    nc.vector.tensor_tensor(out=tmid[1][:], in0=A_t[1][:],
                                            in1=B_t[1][:], op=OP.add)
                    nc.vector.tensor_scalar_mul(tmid[1][:], tmid[1][:], 0.5)
                    acc1, i_nthr = probe_act16_pre(1, tmid[1][:])
                    nc.vector.tensor_tensor(out=tmid[0][:], in0=A_t[0][:],
                                            in1=B_t[0][:], op=OP.add)
                    nc.vector.tensor_scalar_mul(tmid[0][:], tmid[0][:], 0.5)
                    ip = probe_dve16(0, tmid[0][:], c_pr[0][:])
                    if i_nthr_prev is not None:
                        add_dep_helper(ip.ins, i_nthr_prev.ins, sync=False,
                                       reason="no starve")
                    i_nthr_prev = i_nthr
                    probe_act16_post(acc1, c_pr[1][:])
                    bis_update(1, c_pr[1][:], tmid[1][:])
                    bis_update(0, c_pr[0][:], tmid[0][:])

                # finisher: t* = (KTOK - CB)-th largest f32 among {x16 <= B};
                # sel threshold = midpoint between ranks rm1 and rm1+1 (never
                # equals a data value, so the ScalarE Sign sel is exact)
                yband = cp.tile([128, H], F32, tag="yband")
                cnt_t = [cp.tile([128, CH], F32, tag=f"cnt{t}",
                                 name=f"cnt_t{t}") for t in range(NTT)]
                i_cc = []
                i_nmid_prev = None
                for t in range(NTT):
                    ib = nc.vector.scalar_tensor_tensor(yband[:], x16_t[t][:],
                                                        B_t[t][:], x32_t[t][:],
                                                        op0=OP.is_le,
                                                        op1=OP.mult)
                    if i_nmid_prev is not None:
                        # keep tile-0's tiny rank-select chain ahead of
                        # tile-1's 11.6us band pass so sel-t0/AR0 go early
                        add_dep_helper(ib.ins, i_nmid_prev.ins, sync=False,
                                       reason="no starve")
                    m8 = st.tile([128, 8], F32, tag=f"m8{t}")
                    nc.vector.max(out=m8[:], in_=yband[:])
                    rm1 = st.tile([128, 1], F32, tag=f"rm1{t}")
                    nc.vector.tensor_scalar(rm1[:], CB_t[t][:],
                                            float(-(KTOK - 1)), -1.0,
                                            op0=OP.add, op1=OP.mult)
                    rm1p = st.tile([128, 1], F32, tag=f"rm1p{t}")
                    nc.vector.tensor_scalar(rm1p[:], rm1[:], 1.0, None,
                                            op0=OP.add)
                    rm1q = st.tile([128, 1], F32, tag=f"rm1q{t}")
                    nc.vector.tensor_scalar(rm1q[:], rm1[:], 2.0, None,
                                            op0=OP.add)
                    # windowed rank match (robust to +-0.5 CB offset from the
                    # ScalarE sign-count path): pick i = ceil(rm1), and the
                    # next rank for the midpoint
                    sel8 = st.tile([128, 8], F32, tag=f"sel8{t}")
                    nc.vector.scalar_tensor_tensor(sel8[:], io8[:], rm1[:],
                                                   m8[:], op0=OP.is_ge,
                                                   op1=OP.mult)
                    sel8b = st.tile([128, 8], F32, tag=f"sel8b{t}")
                    nc.vector.scalar_tensor_tensor(sel8b[:], io8[:], rm1p[:],
                                                   sel8[:], op0=OP.is_lt,
                                                   op1=OP.mult,
                                                   accum_out=TS_t[t][:])
                    sel8c = st.tile([128, 8], F32, tag=f"sel8c{t}")
                    nc.vector.scalar_tensor_tensor(sel8c[:], io8[:], rm1p[:],
                                                   m8[:], op0=OP.is_ge,
                                                   op1=OP.mult)
                    ts2 = st.tile([128, 1], F32, tag=f"ts2{t}")
                    sel8d = st.tile([128, 8], F32, tag=f"sel8d{t}")
                    nc.vector.scalar_tensor_tensor(sel8d[:], io8[:], rm1q[:],
                                                   sel8c[:], op0=OP.is_lt,
                                                   op1=OP.mult,
                                                   accum_out=ts2[:])
                    # bias = -(TS + TS2)/2 for the Sign sel
                    nmid = st.tile([128, 1], F32, tag=f"nmid{t}")
                    nc.vector.tensor_tensor(out=nmid[:], in0=TS_t[t][:],
                                            in1=ts2[:], op=OP.add)
                    i_nmid_prev = nc.vector.tensor_scalar_mul(nmid[:],
                                                              nmid[:], -0.5)
                    # sel (+-1 valued) on ScalarE, reusing x16_t[t]'s buffer
                    sel = cp.tile([128, H], F16, tag=f"x16{t}",
                                  name=f"sel{t}")
                    nc.scalar.activation(sel[:], x32_t[t][:], AF.Sign,
                                         bias=nmid[:], scale=1.0)
                    for f in range(CH):
                        nc.tensor.matmul(psum_cnt[:, f:f + 1], sel[:, f::CH],
                                         ones16[:], start=True, stop=True)
                    # counts = (sum(+-1) + 128) / 2
                    nc.vector.tensor_scalar(cnt_t[t][:], psum_cnt[:], 128.0,
                                            0.5, op0=OP.add, op1=OP.mult)
                    nc.sync.dma_start(cc_in[t][:], cnt_t[t][:])
                    cc = nc.gpsimd.collective_compute(
                        "AllReduce", OP.add,
                        replica_groups=[[i for i in range(N_CORES)]],
                        ins=[cc_in[t][:].opt()], outs=[cc_out[t][:].opt()],
                    )
                    i_cc.append(cc)

                # mlp library (partition_all_reduce) for phase B; load
                # overlaps the AllReduce wait
                i_lib_m1 = nc.gpsimd.load_library(library_config.mlp)
                for dep in iota_insts:
                    add_dep_helper(i_lib_m1.ins, dep.ins, sync=False,
                                   reason="lib order")
                add_dep_helper(i_lib_m1.ins, i_cc[-1].ins, sync=False,
                               reason="lib order")

                # ---------- phase B: tau bisection + prefix-scan ties ---------
                call = cp.tile([128, CH], F32, tag="call")
                callb = cp.tile([128, CH], F32, tag="callb")
                nc.sync.dma_start(call[:], cc_out[0][:])
                nc.sync.dma_start(callb[:], cc_out[1][:])
                nc.vector.tensor_tensor(out=call[:], in0=call[:],
                                        in1=callb[:], op=OP.add)
                ycnt = cp.tile([16, YF], F32, tag="ycnt")
                ycntb = cp.tile([16, YF], F32, tag="ycntb")
                nc.sync.dma_start(ycnt[:],
                                  cc_out[0][:].rearrange("(a b) c -> a (b c)",
                                                         a=16))
                nc.sync.dma_start(ycntb[:],
                                  cc_out[1][:].rearrange("(a b) c -> a (b c)",
                                                         a=16))
                nc.vector.tensor_tensor(out=ycnt[:], in0=ycnt[:],
                                        in1=ycntb[:], op=OP.add)

                scr86 = cp.tile([128, CH], U8, tag="scr86")
                gpart = st.tile([128, 1], F32, tag="gpart")
                Gb = st.tile([128, 1], F32, tag="Gb")

                lo = st.tile([128, 1], F32, tag="lo")
                hi = st.tile([128, 1], F32, tag="hi")
                Ghi = st.tile([128, 1], F32, tag="Ghi")
                mid = st.tile([128, 1], F32, tag="mid")
                # counts <= S=2048, and tau=2048 is impossible (it would need
                # >= NCORE neurons selected by every token): range [0, 2047]
                TAU_ITERS = 11
                nc.vector.memset(lo[:], -0.5)
                nc.vector.memset(hi[:], 2047.5)
                nc.vector.memset(Ghi[:], 0.0)
                last_par = i_lib_m1
                for it in range(TAU_ITERS):
                    nc.vector.tensor_tensor(out=mid[:], in0=lo[:], in1=hi[:],
                                            op=OP.add)
                    nc.vector.tensor_scalar_mul(mid[:], mid[:], 0.5)
                    nc.vector.tensor_scalar(scr86[:], call[:], mid[:], None,
                                            op0=OP.is_gt, op1=OP.add,
                                            accum_out=gpart[:])
                    par = nc.gpsimd.partition_all_reduce(
                        Gb[:], gpart[:], channels=128,
                        reduce_op=bass_isa.ReduceOp.add)
                    add_dep_helper(par.ins, last_par.ins, sync=False,
                                   reason="lib order")
                    last_par = par
                    nc.vector.tensor_scalar(mge[:], Gb[:], float(NCORE), None,
                                            op0=OP.is_ge)
                    nc.vector.copy_predicated(lo[:], mge[:], mid[:])
                    nc.vector.tensor_scalar(mlt[:], Gb[:], float(NCORE), None,
                                            op0=OP.is_lt)
                    nc.vector.copy_predicated(hi[:], mlt[:], mid[:])
                    nc.vector.copy_predicated(Ghi[:], mlt[:], Gb[:])
                tau = st.tile([128, 1], F32, tag="tau")
                nc.vector.tensor_scalar(tau[:], lo[:], 0.5, None, op0=OP.add)
                rr = st.tile([128, 1], F32, tag="rr")
                nc.vector.tensor_scalar(rr[:], Ghi[:], float(-NCORE), -1.0,
                                        op0=OP.add, op1=OP.mult)

                # ---------- phase C: y build (prefix ties) + sparse_gather ----
                y = cp.tile([16, YF + YP], F32, tag="y")
                ties = cp.tile([16, YF], F32, tag="ties")
                nc.vector.tensor_scalar(ties[:], ycnt[:], tau[:16, :], None,
                                        op0=OP.is_equal)
                zz = cp.tile([16, YF], F32, tag="zz")
                nc.vector.memset(zz[:], 0.0)
                pfx = cp.tile([16, YF], F32, tag="pfx")
                nc.vector.tensor_tensor_scan(pfx[:], ties[:], zz[:], 0.0,
                                             op0=OP.add, op1=OP.add)
                # cross-partition exclusive offsets of tie totals
                pso16 = pss.tile([16, 1], F32, tag="pso16")
                nc.tensor.matmul(pso16[:], ut16[:], pfx[:, YF - 1:YF],
                                 start=True, stop=True)
                off = st.tile([16, 1], F32, tag="off")
                nc.vector.tensor_copy(off[:], pso16[:])
                rnk = cp.tile([16, YF], F32, tag="rnk")
                nc.vector.tensor_scalar(rnk[:], pfx[:], off[:], None,
                                        op0=OP.add)
                msel = cp.tile([16, YF], F32, tag="msel")
                nc.vector.scalar_tensor_tensor(msel[:], rnk[:], rr[:16, :],
                                               ties[:], op0=OP.is_le,
                                               op1=OP.mult)
                c1y = cp.tile([16, YF], F32, tag="c1y")
                nc.vector.tensor_scalar(c1y[:], ycnt[:], tau[:16, :], None,
                                        op0=OP.is_gt)
                nc.vector.tensor_tensor(out=c1y[:], in0=c1y[:], in1=msel[:],
                                        op=OP.add)
                nc.vector.tensor_tensor(out=y[:, :YF], in0=c1y[:], in1=jy1[:],
                                        op=OP.mult)
                nc.vector.tensor_scalar(y[:, :YF], y[:, :YF], -1.0, None,
                                        op0=OP.add)
                pm = cp.tile([16, YP], F32, tag="pm")
                nc.vector.tensor_scalar(pm[:], pv[:], float(H + NPAD - 1),
                                        None, op0=OP.is_le)
                pv1 = cp.tile([16, YP], F32, tag="pv1")
                nc.vector.tensor_scalar(pv1[:], pv[:], 1.0, None, op0=OP.add)
                nc.vector.tensor_tensor(out=y[:, YF:], in0=pm[:], in1=pv1[:],
                                        op=OP.mult)
                nc.vector.tensor_scalar(y[:, YF:], y[:, YF:], -1.0, None,
                                        op0=OP.add)

                comp = cp.tile([16, NCP // 16], F32, tag="comp")
                nfound = st.tile([1, 1], U32, tag="nfound")
                i_lib8 = nc.gpsimd.load_library(library_config.sparse_gather)
                add_dep_helper(i_lib8.ins, last_par.ins, sync=False,
                               reason="lib order")
                i_sg = nc.gpsimd.sparse_gather(comp[:], y[:],
                                               num_found=nfound[:])
                add_dep_helper(i_sg.ins, i_lib8.ins, sync=False,
                               reason="lib order")

                comp16 = cp.tile([16, NCP // 16], I16, tag="comp16")
                nc.vector.tensor_copy(comp16[:], comp[:])
                for r in range(8):
                    nc.sync.dma_start(compR[16 * r:16 * r + 16, :], comp16[:])

            # ---------- phase D: fused gathers + reduced GEMM -----------------
            i_lib3 = nc.gpsimd.load_library(library_config.mlp)
            add_dep_helper(i_lib3.ins, i_sg.ins, sync=False, reason="lib order")

            with tc.tile_pool(name="gemm", bufs=1) as gp, \
                 tc.tile_pool(name="outp", bufs=3) as op_, \
                 tc.tile_pool(name="pso", bufs=1, space="PSUM") as pso:
                xw_t = [gp.tile([128, 1, 2 * DG], F16, tag=f"xw{kt}",
                                name=f"xw{kt}") for kt in range(KT)]
                prev = i_lib3
                n128_reg = nc.gpsimd.to_reg(128)
                for kt in range(KT):
                    ix = compR[:, 8 * kt:8 * kt + 8]
                    g = nc.gpsimd.dma_gather(xw_t[kt][:], xw_d[:], ix,
                                             num_idxs=128,
                                             num_idxs_reg=n128_reg,
                                             elem_size=2 * DG)
                    add_dep_helper(g.ins, prev.ins, sync=False,
                                   reason="lib order")
                    prev = g

                MT = SG // 128
                MH = MT // 2
                for half in range(2):
                    ptiles = [pso.tile([128, 512], F32, tag=f"po{i}",
                                       name=f"po{half}_{i}")
                              for i in range(2 * MH)]
                    for kt in range(KT):
                        for i in range(MH):
                            m = half * MH + i
                            for n in range(2):
                                nc.tensor.matmul(
                                    ptiles[2 * i + n][:],
                                    xw_t[kt][:, 0, 128 * m:128 * (m + 1)],
                                    xw_t[kt][:, 0,
                                             DG + 512 * n:DG + 512 * (n + 1)],
                                    start=(kt == 0), stop=(kt == KT - 1))
                    for i in range(MH):
                        m = half * MH + i
                        outs = op_.tile([128, DG], F32, tag="outs")
                        if i % 2 == 0:
                            nc.vector.tensor_copy(outs[:, :512],
                                                  ptiles[2 * i][:])
                            nc.vector.tensor_copy(outs[:, 512:],
                                                  ptiles[2 * i + 1][:])
                        else:
                            nc.scalar.copy(outs[:, :512], ptiles[2 * i][:])
                            nc.scalar.copy(outs[:, 512:], ptiles[2 * i + 1][:])
                        nc.sync.dma_start(out_d[128 * m:128 * (m + 1), :],
                                          outs[:])

    return nc, d


def _split_excess_waits(nc):
    """This walrus build rejects >1 sync wait on several instruction structs;
    hoist extra waits into single-wait NOPs placed just before, same engine."""
    for f in nc.m.functions:
        for bb in f.blocks:
            newi = []
            changed = False
            for ins in bb.instructions:
                si = ins.sync_info
                maxw = 1
                if si is not None and len(si.on_wait) > maxw:
                    waits = list(si.on_wait)
                    keep = waits[-maxw:]
                    for i, w in enumerate(waits[:-maxw]):
                        nop = mybir.InstNoOp(name=f"{ins.name}-ws{i}")
                        nop.engine = ins.engine
                        nop.sync_info = mybir.SyncInfo(on_wait=[w], on_update=[])
                        newi.append(nop)
                    ins.sync_info = mybir.SyncInfo(
                        on_wait=list(keep), on_update=list(si.on_update))
                    changed = True
                newi.append(ins)
            if changed:
                bb.instructions[:] = newi


_CACHE = {}


def _get_program():
    if "real" not in _CACHE:
        nc, d = build_program()
        # populate .instr bytes for extended gpsimd instructions
        # (sparse_gather, dma_gather, library reload) - raw Bass doesn't
        # run this codegen pass and walrus errors "ISA wrong length" without it
        from concourse.library_overlay import lower_extended_insts
        lower_extended_insts(nc)
        _split_excess_waits(nc)
        _CACHE["real"] = (nc, d)
    return _CACHE["real"]


def make_in_maps(x2d, W, d):
    """Host-side prep: f32+f16 token slices + per-core fused f16 tensor."""
    H, S = d["H"], d["S"]
    HP, SLOC, SG, DG = d["HP"], d["SLOC"], d["SG"], d["DG"]
    xt = x2d.T.astype(np.float16)          # [H, S]
    wt = W.T.astype(np.float16)            # [H, D]
    x16 = x2d.astype(np.float16)
    in_maps = []
    for c in range(N_CORES):
        sa, cb = divmod(c, 4)
        xw = np.zeros((HP, 2 * DG), np.float16)
        xw[:H, :DG] = xt[:, sa * SG:(sa + 1) * SG]
        xw[:H, DG:] = wt[:, cb * DG:(cb + 1) * DG]
        in_maps.append({
            "xs": np.ascontiguousarray(x2d[c * SLOC:(c + 1) * SLOC, :]),
            "xs16": np.ascontiguousarray(x16[c * SLOC:(c + 1) * SLOC, :]),
            "xw": xw,
        })
    return in_maps


def kernel(x, W):
    x = np.asarray(x)
    W = np.asarray(W)
    B, S, H = x.shape
    D = W.shape[0]
    assert (S, H, D) == (REAL["S"], REAL["H"], REAL["D"])
    nc, d = _get_program()
    in_maps = make_in_maps(x.reshape(S, H), W, d)
    res = run_bass_kernel_spmd(nc, in_maps, core_ids=list(range(N_CORES)))
    SG, DG = d["SG"], d["DG"]
    out = np.empty((S, D), np.float32)
    for c in range(N_CORES):
        sa, cb = divmod(c, 4)
        out[sa * SG:(sa + 1) * SG, cb * DG:(cb + 1) * DG] = res.results[c]["out"]
    return out.reshape(B, S, D)
